# revision 24
# baseline (speedup 1.0000x reference)
"""Multi-head attention (RoPE, causal, fp32) on 8 Trainium2 NeuronCores.

Problem: B=2, S=2048, D=2048, H=16 heads (hd=128).
Sharding: DP=2 (batch) x TP=4 (head groups of 4 heads). Core c handles
batch c//4, head group c%4. Each core computes q/k/v projections for its
512 features, RoPE, causal attention, and a partial o_proj against its
512 columns of Wo. The host sums the 4 partial o_proj outputs per batch.

v2 layout strategy (per core):
  - x and Wq/Wk/Wv arrive in bf16 (host-cast): halves input DMA and
    enables FWL fast weight loads. All projection matmuls are bf16
    (fp32 PSUM accumulate); attention and o_proj stay float32r.
  - Single fused pass over x: per 512-seq chunk, q+k matmuls for head
    pair 0/1, then head pair 2/3, then v matmuls — all reusing the same
    x tiles in SBUF (x is read from HBM exactly once). PSUM cycles
    through 4 two-bank slots (pq01/pk01/pq23/pk23/pvA/pvB) so the PE
    never waits on evictions.
  - RoPE applied at eviction (rowswap via SBUF->SBUF DMA, sign baked
    into the host-provided sin table), on VectorE.
  - Attention entirely in transposed space: scoresT[k, q] tiles,
    lhsT=kT slice, rhs=qT chunk. exp fused into PSUM eviction on
    ScalarE. One global software pipeline across ALL (head, q-chunk)
    pairs: score j-pair steps run two steps ahead of the denominator /
    attn@V accumulation, with per-j-pair ex tiles so the pipeline never
    drains at (h,q) boundaries.
  - Causal: only j <= q k-tiles computed; diagonal k-tiles compute only
    the alive (q >= k) column range (partial-N matmuls for score, den
    and attn@V), one shared [128,128] triangular mask multiplied on the
    edge block.
  - Softmax denominator via all-ones [128,128] stationary matmul (k-sum
    pre-broadcast across partitions); 1/denom via one DVE
    reciprocal_approx_fast; normalization folded into attn@V eviction.
  - o_proj weight-stationary, emits the partial TRANSPOSED ([D_out, S])
    in bf16; host sums the 4 per-batch partials in fp32.
"""

import sys

for _p in ("/opt/trn_rl_repo",):
    if _p not in sys.path:
        sys.path.insert(0, _p)

import ml_dtypes
import numpy as np

import concourse.bass as bass
import concourse.bass_isa as bass_isa
import concourse.mybir as mybir
import concourse.tile as tile
from concourse import bacc, bass_utils


# NOTE: the baseline's --enable-ldw-opt=true patch is incompatible with
# bf16 LDWEIGHTS (walrus rejects FWL loads under ldw-opt), so it is not
# used here. LDWEIGHTS issue is hidden under matmul streaming via the
# dual SBUF read ports, so the elision is not needed.

P = 128          # partitions / head dim
S = 2048         # sequence length
D = 2048         # model dim
F = 512          # features per core (4 heads)
H = 4            # heads per core
HD = 128         # head dim
NJ = D // P      # 16 contraction chunks of 128
NQ = S // 512    # 4 query chunks of 512
SCALE = 1.0 / float(np.sqrt(HD))

F32 = mybir.dt.float32
F32R = mybir.dt.float32r
BF16 = mybir.dt.bfloat16
AFT = mybir.ActivationFunctionType


def _r(ap):
    """View an fp32 AP as float32r for full-rate PE matmuls."""
    return ap.bitcast(F32R)


def _proj_phase(tc, xT, wqT, wkT, wvT, cosT, sinT, qT, kT, vN):
    """Fused q/k/v projections + RoPE: one pass over x.

    Per 512-seq chunk, three j-loops (q all heads, k all heads, v) each
    accumulate a 4-bank PSUM tile; the "acc" tag rotates through 2 slots
    (8 banks) so evictions always overlap the next loop's matmuls. At
    s==0 the input DMA is staggered so the q-loop only carries the
    tensors it needs (x chunk + Wq + half of Wk): the PE is compute-
    paced from the first matmul on.
    """
    nc = tc.nc
    # dram views with the j-chunk (128-row blocks) split out: one strided
    # DMA can then load a whole tensor (the Sync engine's ~0.6us per-DMA
    # issue rate is the s==0 bottleneck, not HBM bandwidth)
    xR = xT.rearrange("(j p) s -> p j s", p=P)
    wqR = wqT.rearrange("(j p) f -> p j f", p=P)
    wkR = wkT.rearrange("(j p) f -> p j f", p=P)
    wvR = wvT.rearrange("(j p) f -> p j f", p=P)
    with tc.tile_pool(name="cs", bufs=1) as cspool, \
         tc.tile_pool(name="w", bufs=1) as wpool, \
         tc.tile_pool(name="xs", bufs=2) as xspool, \
         tc.tile_pool(name="rope", bufs=4) as rpool, \
         tc.tile_pool(name="pp", bufs=1, space="PSUM") as pp:
        cos_sb = cspool.tile([P, S], BF16)
        sin_sb = cspool.tile([P, S], BF16)
        wq_sb = wpool.tile([P, NJ, F], BF16)
        wk_sb = wpool.tile([P, NJ, F], BF16)
        wv_sb = wpool.tile([P, NJ, F], BF16)

        xs_tiles = {}

        def load_xs(s):
            xs = xspool.tile([P, NJ, 512], BF16, name=f"xs{s}", tag="xs")
            xs_tiles[s] = xs
            return xs

        xs0 = load_xs(0)
        # per-j DMAs (each dma_start lands on its own hardware ring, so
        # many small transfers run in parallel) split across the three
        # DMA-capable engine queues to parallelize the ~0.6us-per-DMA
        # issue cost: Sync=x, Scalar=wq then wv, GpSimd=cos/sin then wk
        for j in range(NJ):
            nc.sync.dma_start(xs0[:, j, :], xR[:, j, 0:512])
            # j==0 wq rides the Sync queue: the Scalar queue's first DMA
            # sits behind the ~1.3us ACT_TABLE_LOAD preamble
            weng = nc.sync if j == 0 else nc.scalar
            weng.dma_start(wq_sb[:, j, :], wqR[:, j, :])
        nc.gpsimd.dma_start(cos_sb[:], cosT)
        nc.gpsimd.dma_start(sin_sb[:], sinT)
        for j in range(NJ):
            nc.gpsimd.dma_start(wk_sb[:, j, :], wkR[:, j, :])
            nc.scalar.dma_start(wv_sb[:, j, :], wvR[:, j, :])

        def rope(dst, sl, sh, h):
            # dst = dst*cos + rowswap(dst)*sin, in place on the slab (bf16)
            rt = rpool.tile([P, 512], BF16, name=f"rt{sh}_{h}", tag="rt")
            nc.gpsimd.dma_start(rt[0:64, :], dst[64:128, :])
            nc.gpsimd.dma_start(rt[64:128, :], dst[0:64, :])
            nc.vector.tensor_mul(rt[:], rt[:], sin_sb[:, sl])
            nc.vector.tensor_mul(dst, dst, cos_sb[:, sl])
            nc.vector.tensor_add(dst, dst, rt[:])

        for s in range(NQ):
            sl = slice(s * 512, (s + 1) * 512)
            xs = xs_tiles[s]
            # ---- q loop (all 4 heads) ----
            pq = pp.tile([P, H, 512], F32, name=f"pq{s}", tag="acc", bufs=2)
            for j in range(NJ):
                for h in range(H):
                    nc.tensor.matmul(pq[:, h, :],
                                     wq_sb[:, j, h * HD:(h + 1) * HD],
                                     xs[:, j, :],
                                     start=(j == 0), stop=(j == NJ - 1))

            nc.scalar.activation(qT[:, :, sl], pq[:], AFT.Copy)
            for h in range(H):
                rope(qT[:, h, sl], sl, s, f"q{h}")
            # ---- k loop (all 4 heads) ----
            pk = pp.tile([P, H, 512], F32, name=f"pk{s}", tag="acc", bufs=2)
            for j in range(NJ):
                for h in range(H):
                    nc.tensor.matmul(pk[:, h, :],
                                     wk_sb[:, j, h * HD:(h + 1) * HD],
                                     xs[:, j, :],
                                     start=(j == 0), stop=(j == NJ - 1))

            nc.scalar.activation(kT[:, :, sl], pk[:], AFT.Copy)
            for h in range(H):
                rope(kT[:, h, sl], sl, s, f"k{h}")
            # ---- v loop (x chunks become the stationaries) ----
            pv = pp.tile([P, H, 512], F32, name=f"pv{s}", tag="acc", bufs=2)
            for j in range(NJ):
                for st in range(4):
                    nc.tensor.matmul(pv[:, st, :],
                                     xs[:, j, st * P:(st + 1) * P],
                                     wv_sb[:, j, :],
                                     start=(j == 0), stop=(j == NJ - 1))
                if s < NQ - 1:
                    xsn = load_xs(s + 1) if j == 0 else xs_tiles[s + 1]
                    nc.sync.dma_start(xsn[:, j, :],
                                      xR[:, j, (s + 1) * 512:(s + 2) * 512])
            nc.scalar.activation(vN[:, 4 * s:4 * s + 4, :], pv[:], AFT.Copy)


def _attn_phase(tc, mskT, woT, qT, kT, vN, oT, wo_sb, out):
    """Causal attention, one global software pipeline over (h, q, j-pair)."""
    nc = tc.nc
    with tc.tile_pool(name="amsk", bufs=1, side="right") as mpool, \
         tc.tile_pool(name="exp", bufs=5, side="right") as epool, \
         tc.tile_pool(name="attsb", bufs=2, side="right") as apool, \
         tc.tile_pool(name="oev", bufs=3, side="right") as oevp, \
         tc.tile_pool(name="pa", bufs=1, space="PSUM") as pap:
        msk_sb = mpool.tile([P, P], BF16)
        nc.sync.dma_start(msk_sb[:], mskT)
        # all-ones [128,128] stationary: the denominator matmul then yields
        # the k-sum already broadcast across all 128 partitions of PSUM.
        ones_tmp = mpool.tile([P, P], F32)
        nc.vector.memset(ones_tmp[:], 1.0)
        ones_mat = mpool.tile([P, P], BF16)
        nc.vector.tensor_copy(ones_mat[:], ones_tmp[:])
        # prefetch Wo during attention compute (Scalar queue, per head)
        woR = woT.rearrange("(h p) d -> p h d", p=P)
        for h in range(H):
            nc.scalar.dma_start(wo_sb[:, h, :], woR[:, h, :])

        # ---- flattened list of score/acc steps, q-major so each
        # q-chunk's o_proj burst can run as soon as all heads finish ----
        steps = []
        for q in range(NQ):
            for h in range(H):
                jmax = 4 * (q + 1)
                for jj in range(0, jmax, 2):
                    steps.append((h, q, jj, jmax))
        state = {}  # (h,q) -> dict with psum tiles + ex tiles per step

        def score_step(i):
            h, q, jj, jmax = steps[i]
            qh = qT[:, h, :]
            kh = kT[:, h, :]
            ex = epool.tile([P, 2, 512], BF16, name=f"ex{h}_{q}_{jj}", tag="ex")
            psc = pap.tile([P, 2, 512], F32, name=f"psc{h}{q}{jj}",
                           tag="psc", bufs=2)
            offs = []
            for t in range(2):
                j = jj + t
                dd = j - 4 * q
                off = dd * P if dd > 0 else 0
                offs.append(off)
                nc.tensor.matmul(psc[:, t, off:512],
                                 kh[:, j * P:(j + 1) * P],
                                 qh[:, q * 512 + off:(q + 1) * 512],
                                 start=True, stop=True)
            if offs[0] == offs[1]:
                # same width: one fused exp eviction for the pair
                nc.scalar.activation(ex[:, 0:2, offs[0]:512],
                                     psc[:, 0:2, offs[0]:512],
                                     AFT.Exp, scale=SCALE)
            else:
                for t in range(2):
                    nc.scalar.activation(ex[:, t, offs[t]:512],
                                         psc[:, t, offs[t]:512],
                                         AFT.Exp, scale=SCALE)
            # triangular mask on the diagonal 128-block
            for t in range(2):
                j = jj + t
                dd = j - 4 * q
                if dd >= 0:
                    nc.vector.tensor_mul(
                        ex[:, t, dd * P:(dd + 1) * P],
                        ex[:, t, dd * P:(dd + 1) * P], msk_sb[:])
            state[(h, q, jj)] = ex

        def acc_step(i):
            h, q, jj, jmax = steps[i]
            ex = state.pop((h, q, jj))
            key = (h, q)
            if jj == 0:
                pden = pap.tile([P, 512], F32, name=f"pden{h}{q}",
                                tag="pden", bufs=2)
                pov = pap.tile([P, 512], F32, name=f"pov{h}{q}",
                               tag="pov", bufs=2)
                state[key] = (pden, pov)
            pden, pov = state[key]
            for t in range(2):
                j = jj + t
                dd = j - 4 * q
                off = dd * P if dd > 0 else 0
                st = (j == 0)
                sp = (j == jmax - 1)
                # full-array ones matmul: denominator lands pre-broadcast
                # across all 128 partitions
                nc.tensor.matmul(pden[:, off:512], ones_mat[:],
                                 ex[:, t, off:512], start=st, stop=sp)
                nc.tensor.matmul(pov[:, off:512],
                                 vN[:, j, h * HD:(h + 1) * HD],
                                 ex[:, t, off:512], start=st, stop=sp)
            if jj == jmax - 2:
                del state[key]
                rbc = apool.tile([P, 512], F32, name=f"rbc{h}{q}", tag="rbc")
                nc.vector.reciprocal_approx_fast(rbc[:], pden[:])
                nc.vector.tensor_mul(
                    oT[:, h, q * 512:(q + 1) * 512],
                    pov[:], rbc[:])

        def oproj_burst(qc):
            # o_proj for this q-chunk: weight-stationary over 16 dt blocks,
            # PSUM shared with the score tiles ("psc" tag), evictions on
            # VectorE (ScalarE is saturated by exp), bf16 stores
            qsl = slice(qc * 512, (qc + 1) * 512)
            for dt2 in range(8):
                po = pap.tile([P, 2, 512], F32, name=f"po{qc}_{dt2}",
                              tag="psc", bufs=2)
                for t in range(2):
                    dt = dt2 * 2 + t
                    for h in range(H):
                        nc.tensor.matmul(po[:, t, :],
                                         wo_sb[:, h, dt * P:(dt + 1) * P],
                                         oT[:, h, qsl],
                                         start=(h == 0), stop=(h == H - 1))
                ot = oevp.tile([P, 2, 512], BF16, name=f"ot{qc}_{dt2}",
                               tag="ot")
                if qc == NQ - 1 and dt2 % 2 == 1:
                    # last q-chunk: exp is finished, ScalarE is free to
                    # share eviction duty and shorten the tail
                    nc.scalar.activation(ot[:], po[:], AFT.Copy)
                else:
                    nc.vector.tensor_copy(ot[:], po[:])
                # stores alternate between the Sync and Scalar DMA queues
                eng = nc.sync if dt2 % 2 == 0 else nc.scalar
                for t in range(2):
                    dt = dt2 * 2 + t
                    eng.dma_start(out[dt * P:(dt + 1) * P, qsl], ot[:, t, :])

        def acc_and_maybe_burst(i):
            acc_step(i)
            h, q, jj, jmax = steps[i]
            if h == H - 1 and jj == jmax - 2:
                oproj_burst(q)

        # software pipeline: scores two steps ahead of accumulation
        n = len(steps)
        for i in range(n):
            score_step(i)
            if i >= 2:
                acc_and_maybe_burst(i - 2)
        acc_and_maybe_burst(n - 2)
        acc_and_maybe_burst(n - 1)


def _body(tc, xT, wqT, wkT, wvT, woT, cosT, sinT, mskT, out):
    nc = tc.nc
    # long-lived slabs; left stack for qkv, right for attention-era tensors
    p_qk = tc.alloc_tile_pool(name="p_qk", bufs=1, side="left")
    qT = p_qk.tile([P, H, S], BF16)   # [hd, head, seq]
    kT = p_qk.tile([P, H, S], BF16)
    p_v = tc.alloc_tile_pool(name="p_v", bufs=1, side="left")
    vN = p_v.tile([P, NJ, F], BF16)   # [:, j, :] = v[j*128:(j+1)*128, :]

    _proj_phase(tc, xT, wqT, wkT, wvT, cosT, sinT, qT, kT, vN)

    p_oT = tc.alloc_tile_pool(name="p_oT", bufs=1, side="right")
    oT = p_oT.tile([P, H, S], BF16)   # attention output, transposed
    p_wo = tc.alloc_tile_pool(name="p_wo", bufs=1, side="right")
    wo_sb = p_wo.tile([P, H, D], BF16)

    _attn_phase(tc, mskT, woT, qT, kT, vN, oT, wo_sb, out)

    p_v.release()
    p_qk.release()
    p_wo.release()
    p_oT.release()


def build_nc():
    nc = bacc.Bacc("TRN2", target_bir_lowering=False, debug=False,
                   enable_asserts=True, num_devices=8)
    xT = nc.dram_tensor("xT", [D, S], BF16, kind="ExternalInput").ap()
    wqT = nc.dram_tensor("wqT", [D, F], BF16, kind="ExternalInput").ap()
    wkT = nc.dram_tensor("wkT", [D, F], BF16, kind="ExternalInput").ap()
    wvT = nc.dram_tensor("wvT", [D, F], BF16, kind="ExternalInput").ap()
    woT = nc.dram_tensor("woT", [F, D], BF16, kind="ExternalInput").ap()
    cosT = nc.dram_tensor("cosT", [P, S], BF16, kind="ExternalInput").ap()
    sinT = nc.dram_tensor("sinT", [P, S], BF16, kind="ExternalInput").ap()
    mskT = nc.dram_tensor("mskT", [P, P], BF16, kind="ExternalInput").ap()
    out = nc.dram_tensor("out", [S, D], BF16, kind="ExternalOutput").ap()

    with tile.TileContext(nc) as tc:
        _body(tc, xT, wqT, wkT, wvT, woT, cosT, sinT, mskT, out)
    nc.compile()
    return nc


_CACHE = {}


def _get_nc():
    if "nc" not in _CACHE:
        _CACHE["nc"] = build_nc()
    return _CACHE["nc"]


def _rope_tables():
    hd = HD
    inv = 1.0 / (10000.0 ** (np.arange(0, hd, 2, dtype=np.float32) / np.float32(hd)))
    t = np.arange(S, dtype=np.float32)
    freqs = np.outer(t, inv)                      # [S, 64]
    emb = np.concatenate([freqs, freqs], axis=-1)  # [S, 128]
    cosT = np.cos(emb).T.astype(np.float32).copy()
    sinT = np.sin(emb).T.astype(np.float32).copy()
    sinT[0:64, :] *= -1.0  # sign of rotate_half baked into the table
    return np.ascontiguousarray(cosT), np.ascontiguousarray(sinT)


def _diag_masks():
    kp = np.arange(P)[:, None]
    qf = np.arange(P)[None, :]
    return np.ascontiguousarray((kp <= qf).astype(np.float32))


def _in_maps(x, Wq, Wk, Wv, Wo):
    cosT, sinT = _rope_tables()
    msk = _diag_masks()
    BF = ml_dtypes.bfloat16
    maps = []
    for c in range(8):
        b, g = c // 4, c % 4
        fs = slice(g * F, (g + 1) * F)
        maps.append({
            "xT": np.ascontiguousarray(x[b].T).astype(BF),
            "wqT": np.ascontiguousarray(Wq[fs, :].T).astype(BF),
            "wkT": np.ascontiguousarray(Wk[fs, :].T).astype(BF),
            "wvT": np.ascontiguousarray(Wv[fs, :].T).astype(BF),
            "woT": np.ascontiguousarray(Wo[:, fs].T).astype(BF),
            "cosT": cosT.astype(BF),
            "sinT": sinT.astype(BF),
            "mskT": msk.astype(BF),
        })
    return maps


def run(x, Wq, Wk, Wv, Wo, trace=False, **spmd_kwargs):
    """Run on 8 cores; returns (full_output, BassKernelResults)."""
    x = np.asarray(x, np.float32)
    Wq = np.asarray(Wq, np.float32)
    Wk = np.asarray(Wk, np.float32)
    Wv = np.asarray(Wv, np.float32)
    Wo = np.asarray(Wo, np.float32)
    nc = _get_nc()
    maps = _in_maps(x, Wq, Wk, Wv, Wo)
    res = bass_utils.run_bass_kernel_spmd(nc, maps, core_ids=list(range(8)),
                                          trace=trace, **spmd_kwargs)
    outs = [res.results[c]["out"].astype(np.float32) for c in range(8)]
    full = np.empty((2, S, D), np.float32)
    for b in range(2):
        # each core returns its o_proj partial TRANSPOSED ([D_out, S])
        acc = outs[4 * b] + outs[4 * b + 1] + outs[4 * b + 2] + outs[4 * b + 3]
        full[b] = acc.T
    return full, res


def kernel(x, Wq, Wk, Wv, Wo):
    full, _ = run(x, Wq, Wk, Wv, Wo)
    return full


# revision 26
# speedup vs baseline: 1.0044x; 1.0044x over previous
"""Multi-head attention (RoPE, causal, fp32) on 8 Trainium2 NeuronCores.

Problem: B=2, S=2048, D=2048, H=16 heads (hd=128).
Sharding: DP=2 (batch) x TP=4 (head groups of 4 heads). Core c handles
batch c//4, head group c%4. Each core computes q/k/v projections for its
512 features, RoPE, causal attention, and a partial o_proj against its
512 columns of Wo. The host sums the 4 partial o_proj outputs per batch.

v2 layout strategy (per core):
  - x and Wq/Wk/Wv arrive in bf16 (host-cast): halves input DMA and
    enables FWL fast weight loads. All projection matmuls are bf16
    (fp32 PSUM accumulate); attention and o_proj stay float32r.
  - Single fused pass over x: per 512-seq chunk, q+k matmuls for head
    pair 0/1, then head pair 2/3, then v matmuls — all reusing the same
    x tiles in SBUF (x is read from HBM exactly once). PSUM cycles
    through 4 two-bank slots (pq01/pk01/pq23/pk23/pvA/pvB) so the PE
    never waits on evictions.
  - RoPE applied at eviction (rowswap via SBUF->SBUF DMA, sign baked
    into the host-provided sin table), on VectorE.
  - Attention entirely in transposed space: scoresT[k, q] tiles,
    lhsT=kT slice, rhs=qT chunk. exp fused into PSUM eviction on
    ScalarE. One global software pipeline across ALL (head, q-chunk)
    pairs: score j-pair steps run two steps ahead of the denominator /
    attn@V accumulation, with per-j-pair ex tiles so the pipeline never
    drains at (h,q) boundaries.
  - Causal: only j <= q k-tiles computed; diagonal k-tiles compute only
    the alive (q >= k) column range (partial-N matmuls for score, den
    and attn@V), one shared [128,128] triangular mask multiplied on the
    edge block.
  - Softmax denominator via all-ones [128,128] stationary matmul (k-sum
    pre-broadcast across partitions); 1/denom via one DVE
    reciprocal_approx_fast; normalization folded into attn@V eviction.
  - o_proj weight-stationary, emits the partial TRANSPOSED ([D_out, S])
    in bf16; host sums the 4 per-batch partials in fp32.
"""

import sys

for _p in ("/opt/trn_rl_repo",):
    if _p not in sys.path:
        sys.path.insert(0, _p)

import ml_dtypes
import numpy as np

import concourse.bass as bass
import concourse.mybir as mybir
import concourse.tile as tile
from concourse import bacc, bass_utils


# NOTE: the baseline's --enable-ldw-opt=true patch is incompatible with
# bf16 LDWEIGHTS (walrus rejects FWL loads under ldw-opt), so it is not
# used here. LDWEIGHTS issue is hidden under matmul streaming via the
# dual SBUF read ports, so the elision is not needed.

P = 128          # partitions / head dim
S = 2048         # sequence length
D = 2048         # model dim
F = 512          # features per core (4 heads)
H = 4            # heads per core
HD = 128         # head dim
NJ = D // P      # 16 contraction chunks of 128
NQ = S // 512    # 4 query chunks of 512
SCALE = 1.0 / float(np.sqrt(HD))

F32 = mybir.dt.float32
BF16 = mybir.dt.bfloat16
AFT = mybir.ActivationFunctionType


def _proj_phase(tc, xT, wqT, wkT, wvT, cosT, sinT, qT, kT, vN):
    """Fused q/k/v projections + RoPE: one pass over x.

    Per 512-seq chunk, three j-loops (q all heads, k all heads, v) each
    accumulate a 4-bank PSUM tile; the "acc" tag rotates through 2 slots
    (8 banks) so evictions always overlap the next loop's matmuls. At
    s==0 the input DMA is staggered so the q-loop only carries the
    tensors it needs (x chunk + Wq + half of Wk): the PE is compute-
    paced from the first matmul on.
    """
    nc = tc.nc
    # dram views with the j-chunk (128-row blocks) split out: one strided
    # DMA can then load a whole tensor (the Sync engine's ~0.6us per-DMA
    # issue rate is the s==0 bottleneck, not HBM bandwidth)
    xR = xT.rearrange("(j p) s -> p j s", p=P)
    wqR = wqT.rearrange("(j p) f -> p j f", p=P)
    wkR = wkT.rearrange("(j p) f -> p j f", p=P)
    wvR = wvT.rearrange("(j p) f -> p j f", p=P)
    with tc.tile_pool(name="cs", bufs=1) as cspool, \
         tc.tile_pool(name="w", bufs=1) as wpool, \
         tc.tile_pool(name="xs", bufs=2) as xspool, \
         tc.tile_pool(name="rope", bufs=4) as rpool, \
         tc.tile_pool(name="pp", bufs=1, space="PSUM") as pp:
        cos_sb = cspool.tile([P, S], BF16)
        sin_sb = cspool.tile([P, S], BF16)
        wq_sb = wpool.tile([P, NJ, F], BF16)
        wk_sb = wpool.tile([P, NJ, F], BF16)
        wv_sb = wpool.tile([P, NJ, F], BF16)

        xs_tiles = {}

        def load_xs(s):
            xs = xspool.tile([P, NJ, 512], BF16, name=f"xs{s}", tag="xs")
            xs_tiles[s] = xs
            return xs

        xs0 = load_xs(0)
        # per-j DMAs (each dma_start lands on its own hardware ring, so
        # many small transfers run in parallel) split across the three
        # DMA-capable engine queues to parallelize the ~0.6us-per-DMA
        # issue cost: Sync=x, Scalar=wq then wv, GpSimd=cos/sin then wk
        for j in range(NJ):
            nc.sync.dma_start(xs0[:, j, :], xR[:, j, 0:512])
            # j==0 wq rides the Sync queue: the Scalar queue's first DMA
            # sits behind the ~1.3us ACT_TABLE_LOAD preamble
            weng = nc.sync if j == 0 else nc.scalar
            weng.dma_start(wq_sb[:, j, :], wqR[:, j, :])
        nc.gpsimd.dma_start(cos_sb[:], cosT)
        nc.gpsimd.dma_start(sin_sb[:], sinT)
        for j in range(NJ):
            nc.gpsimd.dma_start(wk_sb[:, j, :], wkR[:, j, :])
            nc.scalar.dma_start(wv_sb[:, j, :], wvR[:, j, :])

        def rope(dst, sl, sh, h):
            # dst = dst*cos + rowswap(dst)*sin, in place on the slab (bf16)
            rt = rpool.tile([P, 512], BF16, name=f"rt{sh}_{h}", tag="rt")
            nc.gpsimd.dma_start(rt[0:64, :], dst[64:128, :])
            nc.gpsimd.dma_start(rt[64:128, :], dst[0:64, :])
            nc.vector.tensor_mul(rt[:], rt[:], sin_sb[:, sl])
            nc.vector.tensor_mul(dst, dst, cos_sb[:, sl])
            nc.vector.tensor_add(dst, dst, rt[:])

        for s in range(NQ):
            sl = slice(s * 512, (s + 1) * 512)
            xs = xs_tiles[s]
            # ---- q loop (all 4 heads) ----
            pq = pp.tile([P, H, 512], F32, name=f"pq{s}", tag="acc", bufs=2)
            for j in range(NJ):
                for h in range(H):
                    nc.tensor.matmul(pq[:, h, :],
                                     wq_sb[:, j, h * HD:(h + 1) * HD],
                                     xs[:, j, :],
                                     start=(j == 0), stop=(j == NJ - 1))

            nc.scalar.activation(qT[:, :, sl], pq[:], AFT.Copy)
            for h in range(H):
                rope(qT[:, h, sl], sl, s, f"q{h}")
            # ---- k loop (all 4 heads) ----
            pk = pp.tile([P, H, 512], F32, name=f"pk{s}", tag="acc", bufs=2)
            for j in range(NJ):
                for h in range(H):
                    nc.tensor.matmul(pk[:, h, :],
                                     wk_sb[:, j, h * HD:(h + 1) * HD],
                                     xs[:, j, :],
                                     start=(j == 0), stop=(j == NJ - 1))

            nc.scalar.activation(kT[:, :, sl], pk[:], AFT.Copy)
            for h in range(H):
                rope(kT[:, h, sl], sl, s, f"k{h}")
            # ---- v loop (x chunks become the stationaries) ----
            pv = pp.tile([P, H, 512], F32, name=f"pv{s}", tag="acc", bufs=2)
            for j in range(NJ):
                for st in range(4):
                    nc.tensor.matmul(pv[:, st, :],
                                     xs[:, j, st * P:(st + 1) * P],
                                     wv_sb[:, j, :],
                                     start=(j == 0), stop=(j == NJ - 1))
                if s < NQ - 1:
                    xsn = load_xs(s + 1) if j == 0 else xs_tiles[s + 1]
                    nc.sync.dma_start(xsn[:, j, :],
                                      xR[:, j, (s + 1) * 512:(s + 2) * 512])
            nc.scalar.activation(vN[:, 4 * s:4 * s + 4, :], pv[:], AFT.Copy)


def _attn_phase(tc, mskT, woT, qT, kT, vN, oT, wo_sb, out):
    """Causal attention, one global software pipeline over (h, q, j-pair)."""
    nc = tc.nc
    with tc.tile_pool(name="amsk", bufs=1, side="right") as mpool, \
         tc.tile_pool(name="exp", bufs=5, side="right") as epool, \
         tc.tile_pool(name="attsb", bufs=2, side="right") as apool, \
         tc.tile_pool(name="oev", bufs=3, side="right") as oevp, \
         tc.tile_pool(name="pa", bufs=1, space="PSUM") as pap:
        msk_sb = mpool.tile([P, P], BF16)
        nc.sync.dma_start(msk_sb[:], mskT)
        # all-ones [128,128] stationary: the denominator matmul then yields
        # the k-sum already broadcast across all 128 partitions of PSUM.
        ones_tmp = mpool.tile([P, P], F32)
        nc.vector.memset(ones_tmp[:], 1.0)
        ones_mat = mpool.tile([P, P], BF16)
        nc.vector.tensor_copy(ones_mat[:], ones_tmp[:])
        # prefetch Wo during attention compute (Scalar queue, per head)
        woR = woT.rearrange("(h p) d -> p h d", p=P)
        for h in range(H):
            nc.scalar.dma_start(wo_sb[:, h, :], woR[:, h, :])

        # ---- flattened list of score/acc steps, q-major so each
        # q-chunk's o_proj burst can run as soon as all heads finish ----
        steps = []
        for q in range(NQ):
            for h in range(H):
                jmax = 4 * (q + 1)
                for jj in range(0, jmax, 2):
                    steps.append((h, q, jj, jmax))
        state = {}  # (h,q) -> dict with psum tiles + ex tiles per step

        def score_step(i):
            h, q, jj, jmax = steps[i]
            qh = qT[:, h, :]
            kh = kT[:, h, :]
            ex = epool.tile([P, 2, 512], BF16, name=f"ex{h}_{q}_{jj}", tag="ex")
            psc = pap.tile([P, 2, 512], F32, name=f"psc{h}{q}{jj}",
                           tag="psc", bufs=2)
            offs = []
            for t in range(2):
                j = jj + t
                dd = j - 4 * q
                off = dd * P if dd > 0 else 0
                offs.append(off)
                nc.tensor.matmul(psc[:, t, off:512],
                                 kh[:, j * P:(j + 1) * P],
                                 qh[:, q * 512 + off:(q + 1) * 512],
                                 start=True, stop=True)
            if offs[0] == offs[1]:
                # same width: one fused exp eviction for the pair
                nc.scalar.activation(ex[:, 0:2, offs[0]:512],
                                     psc[:, 0:2, offs[0]:512],
                                     AFT.Exp, scale=SCALE)
            else:
                for t in range(2):
                    nc.scalar.activation(ex[:, t, offs[t]:512],
                                         psc[:, t, offs[t]:512],
                                         AFT.Exp, scale=SCALE)
            # triangular mask on the diagonal 128-block
            for t in range(2):
                j = jj + t
                dd = j - 4 * q
                if dd >= 0:
                    nc.vector.tensor_mul(
                        ex[:, t, dd * P:(dd + 1) * P],
                        ex[:, t, dd * P:(dd + 1) * P], msk_sb[:])
            state[(h, q, jj)] = ex

        def acc_step(i):
            h, q, jj, jmax = steps[i]
            ex = state.pop((h, q, jj))
            key = (h, q)
            if jj == 0:
                pden = pap.tile([P, 512], F32, name=f"pden{h}{q}",
                                tag="pden", bufs=2)
                pov = pap.tile([P, 512], F32, name=f"pov{h}{q}",
                               tag="pov", bufs=2)
                state[key] = (pden, pov)
            pden, pov = state[key]
            for t in range(2):
                j = jj + t
                dd = j - 4 * q
                off = dd * P if dd > 0 else 0
                st = (j == 0)
                sp = (j == jmax - 1)
                # full-array ones matmul: denominator lands pre-broadcast
                # across all 128 partitions
                nc.tensor.matmul(pden[:, off:512], ones_mat[:],
                                 ex[:, t, off:512], start=st, stop=sp)
                nc.tensor.matmul(pov[:, off:512],
                                 vN[:, j, h * HD:(h + 1) * HD],
                                 ex[:, t, off:512], start=st, stop=sp)
            if jj == jmax - 2:
                del state[key]
                rbc = apool.tile([P, 512], F32, name=f"rbc{h}{q}", tag="rbc")
                nc.vector.reciprocal_approx_fast(rbc[:], pden[:])
                nc.vector.tensor_mul(
                    oT[:, h, q * 512:(q + 1) * 512],
                    pov[:], rbc[:])

        def oproj_burst(qc):
            # o_proj for this q-chunk: weight-stationary over 16 dt blocks,
            # PSUM shared with the score tiles ("psc" tag), evictions on
            # VectorE (ScalarE is saturated by exp), bf16 stores
            qsl = slice(qc * 512, (qc + 1) * 512)
            for dt2 in range(8):
                po = pap.tile([P, 2, 512], F32, name=f"po{qc}_{dt2}",
                              tag="psc", bufs=2)
                for t in range(2):
                    dt = dt2 * 2 + t
                    for h in range(H):
                        nc.tensor.matmul(po[:, t, :],
                                         wo_sb[:, h, dt * P:(dt + 1) * P],
                                         oT[:, h, qsl],
                                         start=(h == 0), stop=(h == H - 1))
                ot = oevp.tile([P, 2, 512], BF16, name=f"ot{qc}_{dt2}",
                               tag="ot")
                if qc == NQ - 1 and dt2 % 2 == 1:
                    # last q-chunk: exp is finished, ScalarE is free to
                    # share eviction duty and shorten the tail
                    nc.scalar.activation(ot[:], po[:], AFT.Copy)
                else:
                    nc.vector.tensor_copy(ot[:], po[:])
                # stores alternate between the Sync and Scalar DMA queues
                eng = nc.sync if dt2 % 2 == 0 else nc.scalar
                for t in range(2):
                    dt = dt2 * 2 + t
                    eng.dma_start(out[dt * P:(dt + 1) * P, qsl], ot[:, t, :])

        def acc_and_maybe_burst(i):
            acc_step(i)
            h, q, jj, jmax = steps[i]
            if h == H - 1 and jj == jmax - 2:
                oproj_burst(q)

        # software pipeline: scores two steps ahead of accumulation
        n = len(steps)
        for i in range(n):
            score_step(i)
            if i >= 2:
                acc_and_maybe_burst(i - 2)
        acc_and_maybe_burst(n - 2)
        acc_and_maybe_burst(n - 1)


def _body(tc, xT, wqT, wkT, wvT, woT, cosT, sinT, mskT, out):
    nc = tc.nc
    # long-lived slabs; left stack for qkv, right for attention-era tensors
    p_qk = tc.alloc_tile_pool(name="p_qk", bufs=1, side="left")
    qT = p_qk.tile([P, H, S], BF16)   # [hd, head, seq]
    kT = p_qk.tile([P, H, S], BF16)
    p_v = tc.alloc_tile_pool(name="p_v", bufs=1, side="left")
    vN = p_v.tile([P, NJ, F], BF16)   # [:, j, :] = v[j*128:(j+1)*128, :]

    _proj_phase(tc, xT, wqT, wkT, wvT, cosT, sinT, qT, kT, vN)

    p_oT = tc.alloc_tile_pool(name="p_oT", bufs=1, side="right")
    oT = p_oT.tile([P, H, S], BF16)   # attention output, transposed
    p_wo = tc.alloc_tile_pool(name="p_wo", bufs=1, side="right")
    wo_sb = p_wo.tile([P, H, D], BF16)

    _attn_phase(tc, mskT, woT, qT, kT, vN, oT, wo_sb, out)

    p_v.release()
    p_qk.release()
    p_wo.release()
    p_oT.release()


def build_nc():
    nc = bacc.Bacc("TRN2", target_bir_lowering=False, debug=False,
                   enable_asserts=True, num_devices=8)
    xT = nc.dram_tensor("xT", [D, S], BF16, kind="ExternalInput").ap()
    wqT = nc.dram_tensor("wqT", [D, F], BF16, kind="ExternalInput").ap()
    wkT = nc.dram_tensor("wkT", [D, F], BF16, kind="ExternalInput").ap()
    wvT = nc.dram_tensor("wvT", [D, F], BF16, kind="ExternalInput").ap()
    woT = nc.dram_tensor("woT", [F, D], BF16, kind="ExternalInput").ap()
    cosT = nc.dram_tensor("cosT", [P, S], BF16, kind="ExternalInput").ap()
    sinT = nc.dram_tensor("sinT", [P, S], BF16, kind="ExternalInput").ap()
    mskT = nc.dram_tensor("mskT", [P, P], BF16, kind="ExternalInput").ap()
    out = nc.dram_tensor("out", [S, D], BF16, kind="ExternalOutput").ap()

    with tile.TileContext(nc) as tc:
        _body(tc, xT, wqT, wkT, wvT, woT, cosT, sinT, mskT, out)
    nc.compile()
    return nc


_CACHE = {}


def _get_nc():
    if "nc" not in _CACHE:
        _CACHE["nc"] = build_nc()
    return _CACHE["nc"]


def _rope_tables():
    hd = HD
    inv = 1.0 / (10000.0 ** (np.arange(0, hd, 2, dtype=np.float32) / np.float32(hd)))
    t = np.arange(S, dtype=np.float32)
    freqs = np.outer(t, inv)                      # [S, 64]
    emb = np.concatenate([freqs, freqs], axis=-1)  # [S, 128]
    cosT = np.cos(emb).T.astype(np.float32).copy()
    sinT = np.sin(emb).T.astype(np.float32).copy()
    sinT[0:64, :] *= -1.0  # sign of rotate_half baked into the table
    return np.ascontiguousarray(cosT), np.ascontiguousarray(sinT)


def _diag_masks():
    kp = np.arange(P)[:, None]
    qf = np.arange(P)[None, :]
    return np.ascontiguousarray((kp <= qf).astype(np.float32))


def _in_maps(x, Wq, Wk, Wv, Wo):
    cosT, sinT = _rope_tables()
    msk = _diag_masks()
    BF = ml_dtypes.bfloat16
    maps = []
    for c in range(8):
        b, g = c // 4, c % 4
        fs = slice(g * F, (g + 1) * F)
        maps.append({
            "xT": np.ascontiguousarray(x[b].T).astype(BF),
            "wqT": np.ascontiguousarray(Wq[fs, :].T).astype(BF),
            "wkT": np.ascontiguousarray(Wk[fs, :].T).astype(BF),
            "wvT": np.ascontiguousarray(Wv[fs, :].T).astype(BF),
            "woT": np.ascontiguousarray(Wo[:, fs].T).astype(BF),
            "cosT": cosT.astype(BF),
            "sinT": sinT.astype(BF),
            "mskT": msk.astype(BF),
        })
    return maps


def run(x, Wq, Wk, Wv, Wo, trace=False, **spmd_kwargs):
    """Run on 8 cores; returns (full_output, BassKernelResults)."""
    x = np.asarray(x, np.float32)
    Wq = np.asarray(Wq, np.float32)
    Wk = np.asarray(Wk, np.float32)
    Wv = np.asarray(Wv, np.float32)
    Wo = np.asarray(Wo, np.float32)
    nc = _get_nc()
    maps = _in_maps(x, Wq, Wk, Wv, Wo)
    res = bass_utils.run_bass_kernel_spmd(nc, maps, core_ids=list(range(8)),
                                          trace=trace, **spmd_kwargs)
    outs = [res.results[c]["out"].astype(np.float32) for c in range(8)]
    full = np.empty((2, S, D), np.float32)
    for b in range(2):
        # each core returns its o_proj partial TRANSPOSED ([D_out, S])
        acc = outs[4 * b] + outs[4 * b + 1] + outs[4 * b + 2] + outs[4 * b + 3]
        full[b] = acc.T
    return full, res


def kernel(x, Wq, Wk, Wv, Wo):
    full, _ = run(x, Wq, Wk, Wv, Wo)
    return full


# revision 27
# speedup vs baseline: 1.0098x; 1.0053x over previous
"""Multi-head attention (RoPE, causal, fp32) on 8 Trainium2 NeuronCores.

Problem: B=2, S=2048, D=2048, H=16 heads (hd=128).
Sharding: DP=2 (batch) x TP=4 (head groups of 4 heads). Core c handles
batch c//4, head group c%4. Each core computes q/k/v projections for its
512 features, RoPE, causal attention, and a partial o_proj against its
512 columns of Wo. The host sums the 4 partial o_proj outputs per batch.

Layout strategy (per core), all compute in bf16 with fp32 PSUM
accumulation (rel-err budget 2e-2; measured ~6.5e-3):
  - All inputs host-cast to bf16: halves HBM traffic and enables FWL
    fast weight loads. Per-chunk DMAs are spread across the Sync,
    Scalar and GpSimd engine queues — each dma_start lands on its own
    hardware ring, so many small transfers run in parallel and the
    ~0.6us/DMA issue rate never starves the PE.
  - Single fused pass over x: per 512-seq chunk, three j-loops (q all
    heads, k all heads, v) each accumulate a 4-bank PSUM tile; the
    "acc" tag rotates through 2 slots so evictions (ScalarE) always
    overlap the next loop's matmuls. x is read from HBM exactly once.
  - RoPE applied on the bf16 slabs after eviction (rowswap via
    SBUF->SBUF DMA on the GpSimd queue, sign baked into the host sin
    table), on VectorE.
  - Attention entirely in transposed space: scoresT[k, q] tiles,
    lhsT=kT slice, rhs=qT chunk, exp fused into the PSUM eviction on
    ScalarE. One global software pipeline across ALL (head, q-chunk)
    pairs — score j-pair steps run two steps ahead of the denominator/
    attn@V accumulation, with per-j-pair ex tiles so the pipeline never
    drains at pair boundaries.
  - Causal: only j <= q k-tiles computed; diagonal k-tiles compute only
    the alive (q >= k) column range (partial-N matmuls for score, den,
    attn@V and the exp eviction), one shared [128,128] triangular mask
    multiplied on the edge block.
  - Softmax denominator via all-ones [128,128] stationary matmul (k-sum
    pre-broadcast across partitions); 1/denom via one DVE
    reciprocal_approx_fast; normalization folded into the attn@V
    eviction.
  - Attention is q-chunk-major and the o_proj for each q-chunk runs as
    a burst right after its last head finishes, sharing PSUM with the
    score tiles: output stores start ~60us early and the kernel tail is
    only the last chunk's drain. Partials stored TRANSPOSED ([D_out,S])
    in bf16; host sums the 4 per-batch partials in fp32.

Known-dead ends (measured slower or incorrect on HW): col-tiled 32-wide
denominator strips (tile-mode-switch drains + nondeterministic results
next to fp32r matmuls), GpSimd partition_all_reduce denominators
(~3.7us per [128,512] — serializes each q-block), whole-tensor batched
DMAs (one ring per dma_start caps at ~35GB/s).
"""

import sys

for _p in ("/opt/trn_rl_repo",):
    if _p not in sys.path:
        sys.path.insert(0, _p)

import ml_dtypes
import numpy as np

import concourse.bass as bass
import concourse.mybir as mybir
import concourse.tile as tile
from concourse import bacc, bass_utils


# NOTE: the baseline's --enable-ldw-opt=true patch is incompatible with
# bf16 LDWEIGHTS (walrus rejects FWL loads under ldw-opt), so it is not
# used here. LDWEIGHTS issue is hidden under matmul streaming via the
# dual SBUF read ports, so the elision is not needed.

P = 128          # partitions / head dim
S = 2048         # sequence length
D = 2048         # model dim
F = 512          # features per core (4 heads)
H = 4            # heads per core
HD = 128         # head dim
NJ = D // P      # 16 contraction chunks of 128
NQ = S // 512    # 4 query chunks of 512
SCALE = 1.0 / float(np.sqrt(HD))

F32 = mybir.dt.float32
BF16 = mybir.dt.bfloat16
AFT = mybir.ActivationFunctionType


def _proj_phase(tc, xT, wqT, wkT, wvT, cosT, sinT, qT, kT, vN):
    """Fused q/k/v projections + RoPE: one pass over x.

    Per 512-seq chunk, three j-loops (q all heads, k all heads, v) each
    accumulate a 4-bank PSUM tile; the "acc" tag rotates through 2 slots
    (8 banks) so evictions always overlap the next loop's matmuls. At
    s==0 the input DMA is staggered so the q-loop only carries the
    tensors it needs (x chunk + Wq + half of Wk): the PE is compute-
    paced from the first matmul on.
    """
    nc = tc.nc
    # dram views with the j-chunk (128-row blocks) split out: one strided
    # DMA can then load a whole tensor (the Sync engine's ~0.6us per-DMA
    # issue rate is the s==0 bottleneck, not HBM bandwidth)
    xR = xT.rearrange("(j p) s -> p j s", p=P)
    wqR = wqT.rearrange("(j p) f -> p j f", p=P)
    wkR = wkT.rearrange("(j p) f -> p j f", p=P)
    wvR = wvT.rearrange("(j p) f -> p j f", p=P)
    with tc.tile_pool(name="cs", bufs=1) as cspool, \
         tc.tile_pool(name="w", bufs=1) as wpool, \
         tc.tile_pool(name="xs", bufs=2) as xspool, \
         tc.tile_pool(name="rope", bufs=4) as rpool, \
         tc.tile_pool(name="pp", bufs=1, space="PSUM") as pp:
        cos_sb = cspool.tile([P, S], BF16)
        sin_sb = cspool.tile([P, S], BF16)
        wq_sb = wpool.tile([P, NJ, F], BF16)
        wk_sb = wpool.tile([P, NJ, F], BF16)
        wv_sb = wpool.tile([P, NJ, F], BF16)

        xs_tiles = {}

        def load_xs(s):
            xs = xspool.tile([P, NJ, 512], BF16, name=f"xs{s}", tag="xs")
            xs_tiles[s] = xs
            return xs

        xs0 = load_xs(0)
        # per-j DMAs (each dma_start lands on its own hardware ring, so
        # many small transfers run in parallel) split across the three
        # DMA-capable engine queues to parallelize the ~0.6us-per-DMA
        # issue cost: Sync=x, Scalar=wq then wv, GpSimd=cos/sin then wk
        for j in range(NJ):
            nc.sync.dma_start(xs0[:, j, :], xR[:, j, 0:512])
            # j==0 wq rides the Sync queue: the Scalar queue's first DMA
            # sits behind the ~1.3us ACT_TABLE_LOAD preamble
            weng = nc.sync if j == 0 else nc.scalar
            weng.dma_start(wq_sb[:, j, :], wqR[:, j, :])
        nc.gpsimd.dma_start(cos_sb[:], cosT)
        nc.gpsimd.dma_start(sin_sb[:], sinT)
        for j in range(NJ):
            nc.gpsimd.dma_start(wk_sb[:, j, :], wkR[:, j, :])
            nc.scalar.dma_start(wv_sb[:, j, :], wvR[:, j, :])

        def rope(dst, sl, sh, h):
            # dst = dst*cos + rowswap(dst)*sin, in place on the slab (bf16)
            rt = rpool.tile([P, 512], BF16, name=f"rt{sh}_{h}", tag="rt")
            nc.gpsimd.dma_start(rt[0:64, :], dst[64:128, :])
            nc.gpsimd.dma_start(rt[64:128, :], dst[0:64, :])
            nc.vector.tensor_mul(rt[:], rt[:], sin_sb[:, sl])
            nc.vector.tensor_mul(dst, dst, cos_sb[:, sl])
            nc.vector.tensor_add(dst, dst, rt[:])

        for s in range(NQ):
            sl = slice(s * 512, (s + 1) * 512)
            xs = xs_tiles[s]
            # ---- q loop (all 4 heads) ----
            pq = pp.tile([P, H, 512], F32, name=f"pq{s}", tag="acc", bufs=2)
            for j in range(NJ):
                for h in range(H):
                    nc.tensor.matmul(pq[:, h, :],
                                     wq_sb[:, j, h * HD:(h + 1) * HD],
                                     xs[:, j, :],
                                     start=(j == 0), stop=(j == NJ - 1))

            nc.scalar.activation(qT[:, :, sl], pq[:], AFT.Copy)
            for h in range(H):
                rope(qT[:, h, sl], sl, s, f"q{h}")
            # ---- k loop (all 4 heads) ----
            pk = pp.tile([P, H, 512], F32, name=f"pk{s}", tag="acc", bufs=2)
            for j in range(NJ):
                for h in range(H):
                    nc.tensor.matmul(pk[:, h, :],
                                     wk_sb[:, j, h * HD:(h + 1) * HD],
                                     xs[:, j, :],
                                     start=(j == 0), stop=(j == NJ - 1))

            nc.scalar.activation(kT[:, :, sl], pk[:], AFT.Copy)
            for h in range(H):
                rope(kT[:, h, sl], sl, s, f"k{h}")
            # ---- v loop (x chunks become the stationaries) ----
            pv = pp.tile([P, H, 512], F32, name=f"pv{s}", tag="acc", bufs=2)
            for j in range(NJ):
                for st in range(4):
                    nc.tensor.matmul(pv[:, st, :],
                                     xs[:, j, st * P:(st + 1) * P],
                                     wv_sb[:, j, :],
                                     start=(j == 0), stop=(j == NJ - 1))
                if s < NQ - 1:
                    xsn = load_xs(s + 1) if j == 0 else xs_tiles[s + 1]
                    nc.sync.dma_start(xsn[:, j, :],
                                      xR[:, j, (s + 1) * 512:(s + 2) * 512])
            nc.scalar.activation(vN[:, 4 * s:4 * s + 4, :], pv[:], AFT.Copy)


def _attn_phase(tc, mskT, woT, qT, kT, vN, oT, wo_sb, out):
    """Causal attention, one global software pipeline over (h, q, j-pair)."""
    nc = tc.nc
    with tc.tile_pool(name="amsk", bufs=1, side="right") as mpool, \
         tc.tile_pool(name="exp", bufs=5, side="right") as epool, \
         tc.tile_pool(name="attsb", bufs=2, side="right") as apool, \
         tc.tile_pool(name="oev", bufs=3, side="right") as oevp, \
         tc.tile_pool(name="pa", bufs=1, space="PSUM") as pap:
        msk_sb = mpool.tile([P, P], BF16)
        nc.sync.dma_start(msk_sb[:], mskT)
        # all-ones [128,128] stationary: the denominator matmul then yields
        # the k-sum already broadcast across all 128 partitions of PSUM.
        ones_tmp = mpool.tile([P, P], F32)
        nc.vector.memset(ones_tmp[:], 1.0)
        ones_mat = mpool.tile([P, P], BF16)
        nc.vector.tensor_copy(ones_mat[:], ones_tmp[:])
        # prefetch Wo during attention compute (Scalar queue, per head)
        woR = woT.rearrange("(h p) d -> p h d", p=P)
        for h in range(H):
            nc.scalar.dma_start(wo_sb[:, h, :], woR[:, h, :])

        # ---- flattened list of score/acc steps, q-major so each
        # q-chunk's o_proj burst can run as soon as all heads finish ----
        steps = []
        for q in range(NQ):
            for h in range(H):
                jmax = 4 * (q + 1)
                for jj in range(0, jmax, 2):
                    steps.append((h, q, jj, jmax))
        state = {}  # (h,q) -> dict with psum tiles + ex tiles per step

        def score_step(i):
            h, q, jj, jmax = steps[i]
            qh = qT[:, h, :]
            kh = kT[:, h, :]
            ex = epool.tile([P, 2, 512], BF16, name=f"ex{h}_{q}_{jj}", tag="ex")
            psc = pap.tile([P, 2, 512], F32, name=f"psc{h}{q}{jj}",
                           tag="psc", bufs=2)
            offs = []
            for t in range(2):
                j = jj + t
                dd = j - 4 * q
                off = dd * P if dd > 0 else 0
                offs.append(off)
                nc.tensor.matmul(psc[:, t, off:512],
                                 kh[:, j * P:(j + 1) * P],
                                 qh[:, q * 512 + off:(q + 1) * 512],
                                 start=True, stop=True)
            if offs[0] == offs[1]:
                # same width: one fused exp eviction for the pair
                nc.scalar.activation(ex[:, 0:2, offs[0]:512],
                                     psc[:, 0:2, offs[0]:512],
                                     AFT.Exp, scale=SCALE)
            else:
                for t in range(2):
                    nc.scalar.activation(ex[:, t, offs[t]:512],
                                         psc[:, t, offs[t]:512],
                                         AFT.Exp, scale=SCALE)
            # triangular mask on the diagonal 128-block
            for t in range(2):
                j = jj + t
                dd = j - 4 * q
                if dd >= 0:
                    nc.vector.tensor_mul(
                        ex[:, t, dd * P:(dd + 1) * P],
                        ex[:, t, dd * P:(dd + 1) * P], msk_sb[:])
            state[(h, q, jj)] = ex

        def acc_step(i):
            h, q, jj, jmax = steps[i]
            ex = state.pop((h, q, jj))
            key = (h, q)
            if jj == 0:
                pden = pap.tile([P, 512], F32, name=f"pden{h}{q}",
                                tag="pden", bufs=2)
                pov = pap.tile([P, 512], F32, name=f"pov{h}{q}",
                               tag="pov", bufs=2)
                state[key] = (pden, pov)
            pden, pov = state[key]
            for t in range(2):
                j = jj + t
                dd = j - 4 * q
                off = dd * P if dd > 0 else 0
                st = (j == 0)
                sp = (j == jmax - 1)
                # full-array ones matmul: denominator lands pre-broadcast
                # across all 128 partitions
                nc.tensor.matmul(pden[:, off:512], ones_mat[:],
                                 ex[:, t, off:512], start=st, stop=sp)
                nc.tensor.matmul(pov[:, off:512],
                                 vN[:, j, h * HD:(h + 1) * HD],
                                 ex[:, t, off:512], start=st, stop=sp)
            if jj == jmax - 2:
                del state[key]
                rbc = apool.tile([P, 512], F32, name=f"rbc{h}{q}", tag="rbc")
                nc.vector.reciprocal_approx_fast(rbc[:], pden[:])
                nc.vector.tensor_mul(
                    oT[:, h, q * 512:(q + 1) * 512],
                    pov[:], rbc[:])

        def oproj_burst(qc):
            # o_proj for this q-chunk: weight-stationary over 16 dt blocks,
            # PSUM shared with the score tiles ("psc" tag), evictions on
            # VectorE (ScalarE is saturated by exp), bf16 stores
            qsl = slice(qc * 512, (qc + 1) * 512)
            for dt2 in range(8):
                po = pap.tile([P, 2, 512], F32, name=f"po{qc}_{dt2}",
                              tag="psc", bufs=2)
                for t in range(2):
                    dt = dt2 * 2 + t
                    for h in range(H):
                        nc.tensor.matmul(po[:, t, :],
                                         wo_sb[:, h, dt * P:(dt + 1) * P],
                                         oT[:, h, qsl],
                                         start=(h == 0), stop=(h == H - 1))
                ot = oevp.tile([P, 2, 512], BF16, name=f"ot{qc}_{dt2}",
                               tag="ot")
                if qc == NQ - 1 and dt2 % 2 == 1:
                    # last q-chunk: exp is finished, ScalarE is free to
                    # share eviction duty and shorten the tail
                    nc.scalar.activation(ot[:], po[:], AFT.Copy)
                else:
                    nc.vector.tensor_copy(ot[:], po[:])
                # stores alternate between the Sync and Scalar DMA queues
                eng = nc.sync if dt2 % 2 == 0 else nc.scalar
                for t in range(2):
                    dt = dt2 * 2 + t
                    eng.dma_start(out[dt * P:(dt + 1) * P, qsl], ot[:, t, :])

        def acc_and_maybe_burst(i):
            acc_step(i)
            h, q, jj, jmax = steps[i]
            if h == H - 1 and jj == jmax - 2:
                oproj_burst(q)

        # software pipeline: scores two steps ahead of accumulation
        n = len(steps)
        for i in range(n):
            score_step(i)
            if i >= 2:
                acc_and_maybe_burst(i - 2)
        acc_and_maybe_burst(n - 2)
        acc_and_maybe_burst(n - 1)


def _body(tc, xT, wqT, wkT, wvT, woT, cosT, sinT, mskT, out):
    nc = tc.nc
    # long-lived slabs; left stack for qkv, right for attention-era tensors
    p_qk = tc.alloc_tile_pool(name="p_qk", bufs=1, side="left")
    qT = p_qk.tile([P, H, S], BF16)   # [hd, head, seq]
    kT = p_qk.tile([P, H, S], BF16)
    p_v = tc.alloc_tile_pool(name="p_v", bufs=1, side="left")
    vN = p_v.tile([P, NJ, F], BF16)   # [:, j, :] = v[j*128:(j+1)*128, :]

    _proj_phase(tc, xT, wqT, wkT, wvT, cosT, sinT, qT, kT, vN)

    p_oT = tc.alloc_tile_pool(name="p_oT", bufs=1, side="right")
    oT = p_oT.tile([P, H, S], BF16)   # attention output, transposed
    p_wo = tc.alloc_tile_pool(name="p_wo", bufs=1, side="right")
    wo_sb = p_wo.tile([P, H, D], BF16)

    _attn_phase(tc, mskT, woT, qT, kT, vN, oT, wo_sb, out)

    p_v.release()
    p_qk.release()
    p_wo.release()
    p_oT.release()


def build_nc():
    nc = bacc.Bacc("TRN2", target_bir_lowering=False, debug=False,
                   enable_asserts=True, num_devices=8)
    xT = nc.dram_tensor("xT", [D, S], BF16, kind="ExternalInput").ap()
    wqT = nc.dram_tensor("wqT", [D, F], BF16, kind="ExternalInput").ap()
    wkT = nc.dram_tensor("wkT", [D, F], BF16, kind="ExternalInput").ap()
    wvT = nc.dram_tensor("wvT", [D, F], BF16, kind="ExternalInput").ap()
    woT = nc.dram_tensor("woT", [F, D], BF16, kind="ExternalInput").ap()
    cosT = nc.dram_tensor("cosT", [P, S], BF16, kind="ExternalInput").ap()
    sinT = nc.dram_tensor("sinT", [P, S], BF16, kind="ExternalInput").ap()
    mskT = nc.dram_tensor("mskT", [P, P], BF16, kind="ExternalInput").ap()
    out = nc.dram_tensor("out", [S, D], BF16, kind="ExternalOutput").ap()

    with tile.TileContext(nc) as tc:
        _body(tc, xT, wqT, wkT, wvT, woT, cosT, sinT, mskT, out)
    nc.compile()
    return nc


_CACHE = {}


def _get_nc():
    if "nc" not in _CACHE:
        _CACHE["nc"] = build_nc()
    return _CACHE["nc"]


def _rope_tables():
    hd = HD
    inv = 1.0 / (10000.0 ** (np.arange(0, hd, 2, dtype=np.float32) / np.float32(hd)))
    t = np.arange(S, dtype=np.float32)
    freqs = np.outer(t, inv)                      # [S, 64]
    emb = np.concatenate([freqs, freqs], axis=-1)  # [S, 128]
    cosT = np.cos(emb).T.astype(np.float32).copy()
    sinT = np.sin(emb).T.astype(np.float32).copy()
    sinT[0:64, :] *= -1.0  # sign of rotate_half baked into the table
    return np.ascontiguousarray(cosT), np.ascontiguousarray(sinT)


def _diag_masks():
    kp = np.arange(P)[:, None]
    qf = np.arange(P)[None, :]
    return np.ascontiguousarray((kp <= qf).astype(np.float32))


def _in_maps(x, Wq, Wk, Wv, Wo):
    cosT, sinT = _rope_tables()
    msk = _diag_masks()
    BF = ml_dtypes.bfloat16
    maps = []
    for c in range(8):
        b, g = c // 4, c % 4
        fs = slice(g * F, (g + 1) * F)
        maps.append({
            "xT": np.ascontiguousarray(x[b].T).astype(BF),
            "wqT": np.ascontiguousarray(Wq[fs, :].T).astype(BF),
            "wkT": np.ascontiguousarray(Wk[fs, :].T).astype(BF),
            "wvT": np.ascontiguousarray(Wv[fs, :].T).astype(BF),
            "woT": np.ascontiguousarray(Wo[:, fs].T).astype(BF),
            "cosT": cosT.astype(BF),
            "sinT": sinT.astype(BF),
            "mskT": msk.astype(BF),
        })
    return maps


def run(x, Wq, Wk, Wv, Wo, trace=False, **spmd_kwargs):
    """Run on 8 cores; returns (full_output, BassKernelResults)."""
    x = np.asarray(x, np.float32)
    Wq = np.asarray(Wq, np.float32)
    Wk = np.asarray(Wk, np.float32)
    Wv = np.asarray(Wv, np.float32)
    Wo = np.asarray(Wo, np.float32)
    nc = _get_nc()
    maps = _in_maps(x, Wq, Wk, Wv, Wo)
    res = bass_utils.run_bass_kernel_spmd(nc, maps, core_ids=list(range(8)),
                                          trace=trace, **spmd_kwargs)
    outs = [res.results[c]["out"].astype(np.float32) for c in range(8)]
    full = np.empty((2, S, D), np.float32)
    for b in range(2):
        # each core returns its o_proj partial TRANSPOSED ([D_out, S])
        acc = outs[4 * b] + outs[4 * b + 1] + outs[4 * b + 2] + outs[4 * b + 3]
        full[b] = acc.T
    return full, res


def kernel(x, Wq, Wk, Wv, Wo):
    full, _ = run(x, Wq, Wk, Wv, Wo)
    return full


# revision 28
# speedup vs baseline: 1.0119x; 1.0021x over previous
"""Multi-head attention (RoPE, causal, fp32) on 8 Trainium2 NeuronCores.

Problem: B=2, S=2048, D=2048, H=16 heads (hd=128).
Sharding: DP=2 (batch) x TP=4 (head groups of 4 heads). Core c handles
batch c//4, head group c%4. Each core computes q/k/v projections for its
512 features, RoPE, causal attention, and a partial o_proj against its
512 columns of Wo. The host sums the 4 partial o_proj outputs per batch.

Layout strategy (per core), all compute in bf16 with fp32 PSUM
accumulation (rel-err budget 2e-2; measured ~6.5e-3):
  - All inputs host-cast to bf16: halves HBM traffic and enables FWL
    fast weight loads. Per-chunk DMAs are spread across the Sync,
    Scalar and GpSimd engine queues — each dma_start lands on its own
    hardware ring, so many small transfers run in parallel and the
    ~0.6us/DMA issue rate never starves the PE.
  - Single fused pass over x: per 512-seq chunk, three j-loops (q all
    heads, k all heads, v) each accumulate a 4-bank PSUM tile; the
    "acc" tag rotates through 2 slots so evictions (ScalarE) always
    overlap the next loop's matmuls. x is read from HBM exactly once.
  - RoPE applied on the bf16 slabs after eviction (rowswap via
    SBUF->SBUF DMA on the GpSimd queue, sign baked into the host sin
    table), on VectorE.
  - Attention entirely in transposed space: scoresT[k, q] tiles,
    lhsT=kT slice, rhs=qT chunk, exp fused into the PSUM eviction on
    ScalarE. One global software pipeline across ALL (head, q-chunk)
    pairs — score j-pair steps run two steps ahead of the denominator/
    attn@V accumulation, with per-j-pair ex tiles so the pipeline never
    drains at pair boundaries.
  - Causal: only j <= q k-tiles computed; diagonal k-tiles compute only
    the alive (q >= k) column range (partial-N matmuls for score, den,
    attn@V and the exp eviction), one shared [128,128] triangular mask
    multiplied on the edge block.
  - Softmax denominator via all-ones [128,128] stationary matmul (k-sum
    pre-broadcast across partitions); 1/denom via one DVE
    reciprocal_approx_fast; normalization folded into the attn@V
    eviction.
  - Attention is q-chunk-major and the o_proj for each q-chunk runs as
    a burst right after its last head finishes, sharing PSUM with the
    score tiles: output stores start ~60us early and the kernel tail is
    only the last chunk's drain. Partials stored TRANSPOSED ([D_out,S])
    in bf16; host sums the 4 per-batch partials in fp32.

Known-dead ends (measured slower or incorrect on HW): col-tiled 32-wide
denominator strips (tile-mode-switch drains + nondeterministic results
next to fp32r matmuls), GpSimd partition_all_reduce denominators
(~3.7us per [128,512] — serializes each q-block), whole-tensor batched
DMAs (one ring per dma_start caps at ~35GB/s).
"""

import sys

for _p in ("/opt/trn_rl_repo",):
    if _p not in sys.path:
        sys.path.insert(0, _p)

import ml_dtypes
import numpy as np

import concourse.bass as bass
import concourse.mybir as mybir
import concourse.tile as tile
from concourse import bacc, bass_utils


# NOTE: the baseline's --enable-ldw-opt=true patch is incompatible with
# bf16 LDWEIGHTS (walrus rejects FWL loads under ldw-opt), so it is not
# used here. LDWEIGHTS issue is hidden under matmul streaming via the
# dual SBUF read ports, so the elision is not needed.

P = 128          # partitions / head dim
S = 2048         # sequence length
D = 2048         # model dim
F = 512          # features per core (4 heads)
H = 4            # heads per core
HD = 128         # head dim
NJ = D // P      # 16 contraction chunks of 128
NQ = S // 512    # 4 query chunks of 512
SCALE = 1.0 / float(np.sqrt(HD))

F32 = mybir.dt.float32
BF16 = mybir.dt.bfloat16
AFT = mybir.ActivationFunctionType


def _proj_phase(tc, xT, wqT, wkT, wvT, cosT, sinT, qT, kT, vN):
    """Fused q/k/v projections + RoPE: one pass over x.

    Per 512-seq chunk, three j-loops (q all heads, k all heads, v) each
    accumulate a 4-bank PSUM tile; the "acc" tag rotates through 2 slots
    (8 banks) so evictions always overlap the next loop's matmuls. At
    s==0 the input DMA is staggered so the q-loop only carries the
    tensors it needs (x chunk + Wq + half of Wk): the PE is compute-
    paced from the first matmul on.
    """
    nc = tc.nc
    # dram views with the j-chunk (128-row blocks) split out: one strided
    # DMA can then load a whole tensor (the Sync engine's ~0.6us per-DMA
    # issue rate is the s==0 bottleneck, not HBM bandwidth)
    xR = xT.rearrange("(j p) s -> p j s", p=P)
    wqR = wqT.rearrange("(j p) f -> p j f", p=P)
    wkR = wkT.rearrange("(j p) f -> p j f", p=P)
    wvR = wvT.rearrange("(j p) f -> p j f", p=P)
    with tc.tile_pool(name="cs", bufs=1) as cspool, \
         tc.tile_pool(name="w", bufs=1) as wpool, \
         tc.tile_pool(name="xs", bufs=2) as xspool, \
         tc.tile_pool(name="rope", bufs=4) as rpool, \
         tc.tile_pool(name="pp", bufs=1, space="PSUM") as pp:
        cos_sb = cspool.tile([P, S], BF16)
        sin_sb = cspool.tile([P, S], BF16)
        wq_sb = wpool.tile([P, NJ, F], BF16)
        wk_sb = wpool.tile([P, NJ, F], BF16)
        wv_sb = wpool.tile([P, NJ, F], BF16)

        xs_tiles = {}

        def load_xs(s):
            xs = xspool.tile([P, NJ, 512], BF16, name=f"xs{s}", tag="xs")
            xs_tiles[s] = xs
            return xs

        xs0 = load_xs(0)
        # per-j DMAs (each dma_start lands on its own hardware ring, so
        # many small transfers run in parallel) split across the three
        # DMA-capable engine queues to parallelize the ~0.6us-per-DMA
        # issue cost: Sync=x, Scalar=wq then wv, GpSimd=cos/sin then wk
        for j in range(NJ):
            xeng = nc.sync if j % 2 == 0 else nc.gpsimd
            xeng.dma_start(xs0[:, j, :], xR[:, j, 0:512])
            # j==0 wq rides the Sync queue: the Scalar queue's first DMA
            # sits behind the ~1.3us ACT_TABLE_LOAD preamble
            weng = nc.sync if j == 0 else nc.scalar
            weng.dma_start(wq_sb[:, j, :], wqR[:, j, :])
        nc.gpsimd.dma_start(cos_sb[:], cosT)
        nc.gpsimd.dma_start(sin_sb[:], sinT)
        for j in range(NJ):
            nc.gpsimd.dma_start(wk_sb[:, j, :], wkR[:, j, :])
            nc.scalar.dma_start(wv_sb[:, j, :], wvR[:, j, :])

        def rope(dst, sl, sh, h):
            # dst = dst*cos + rowswap(dst)*sin, in place on the slab (bf16)
            rt = rpool.tile([P, 512], BF16, name=f"rt{sh}_{h}", tag="rt")
            nc.gpsimd.dma_start(rt[0:64, :], dst[64:128, :])
            nc.gpsimd.dma_start(rt[64:128, :], dst[0:64, :])
            nc.vector.tensor_mul(rt[:], rt[:], sin_sb[:, sl])
            nc.vector.tensor_mul(dst, dst, cos_sb[:, sl])
            nc.vector.tensor_add(dst, dst, rt[:])

        for s in range(NQ):
            sl = slice(s * 512, (s + 1) * 512)
            xs = xs_tiles[s]
            # ---- q loop (all 4 heads) ----
            pq = pp.tile([P, H, 512], F32, name=f"pq{s}", tag="acc", bufs=2)
            for j in range(NJ):
                for h in range(H):
                    nc.tensor.matmul(pq[:, h, :],
                                     wq_sb[:, j, h * HD:(h + 1) * HD],
                                     xs[:, j, :],
                                     start=(j == 0), stop=(j == NJ - 1))

            nc.scalar.activation(qT[:, :, sl], pq[:], AFT.Copy)
            for h in range(H):
                rope(qT[:, h, sl], sl, s, f"q{h}")
            # ---- k loop (all 4 heads) ----
            pk = pp.tile([P, H, 512], F32, name=f"pk{s}", tag="acc", bufs=2)
            for j in range(NJ):
                for h in range(H):
                    nc.tensor.matmul(pk[:, h, :],
                                     wk_sb[:, j, h * HD:(h + 1) * HD],
                                     xs[:, j, :],
                                     start=(j == 0), stop=(j == NJ - 1))

            nc.scalar.activation(kT[:, :, sl], pk[:], AFT.Copy)
            for h in range(H):
                rope(kT[:, h, sl], sl, s, f"k{h}")
            # ---- v loop (x chunks become the stationaries) ----
            pv = pp.tile([P, H, 512], F32, name=f"pv{s}", tag="acc", bufs=2)
            for j in range(NJ):
                for st in range(4):
                    nc.tensor.matmul(pv[:, st, :],
                                     xs[:, j, st * P:(st + 1) * P],
                                     wv_sb[:, j, :],
                                     start=(j == 0), stop=(j == NJ - 1))
                if s < NQ - 1:
                    xsn = load_xs(s + 1) if j == 0 else xs_tiles[s + 1]
                    nc.sync.dma_start(xsn[:, j, :],
                                      xR[:, j, (s + 1) * 512:(s + 2) * 512])
            nc.scalar.activation(vN[:, 4 * s:4 * s + 4, :], pv[:], AFT.Copy)


def _attn_phase(tc, mskT, woT, qT, kT, vN, oT, wo_sb, out):
    """Causal attention, one global software pipeline over (h, q, j-pair)."""
    nc = tc.nc
    with tc.tile_pool(name="amsk", bufs=1, side="right") as mpool, \
         tc.tile_pool(name="exp", bufs=5, side="right") as epool, \
         tc.tile_pool(name="attsb", bufs=2, side="right") as apool, \
         tc.tile_pool(name="oev", bufs=3, side="right") as oevp, \
         tc.tile_pool(name="pa", bufs=1, space="PSUM") as pap:
        msk_sb = mpool.tile([P, P], BF16)
        nc.sync.dma_start(msk_sb[:], mskT)
        # all-ones [128,128] stationary: the denominator matmul then yields
        # the k-sum already broadcast across all 128 partitions of PSUM.
        ones_tmp = mpool.tile([P, P], F32)
        nc.vector.memset(ones_tmp[:], 1.0)
        ones_mat = mpool.tile([P, P], BF16)
        nc.vector.tensor_copy(ones_mat[:], ones_tmp[:])
        # prefetch Wo during attention compute (Scalar queue, per head)
        woR = woT.rearrange("(h p) d -> p h d", p=P)
        for h in range(H):
            nc.scalar.dma_start(wo_sb[:, h, :], woR[:, h, :])

        # ---- flattened list of score/acc steps, q-major so each
        # q-chunk's o_proj burst can run as soon as all heads finish ----
        steps = []
        for q in range(NQ):
            for h in range(H):
                jmax = 4 * (q + 1)
                for jj in range(0, jmax, 2):
                    steps.append((h, q, jj, jmax))
        state = {}  # (h,q) -> dict with psum tiles + ex tiles per step

        def score_step(i):
            h, q, jj, jmax = steps[i]
            qh = qT[:, h, :]
            kh = kT[:, h, :]
            ex = epool.tile([P, 2, 512], BF16, name=f"ex{h}_{q}_{jj}", tag="ex")
            psc = pap.tile([P, 2, 512], F32, name=f"psc{h}{q}{jj}",
                           tag="psc", bufs=2)
            offs = []
            for t in range(2):
                j = jj + t
                dd = j - 4 * q
                off = dd * P if dd > 0 else 0
                offs.append(off)
                nc.tensor.matmul(psc[:, t, off:512],
                                 kh[:, j * P:(j + 1) * P],
                                 qh[:, q * 512 + off:(q + 1) * 512],
                                 start=True, stop=True)
            if offs[0] == offs[1]:
                # same width: one fused exp eviction for the pair
                nc.scalar.activation(ex[:, 0:2, offs[0]:512],
                                     psc[:, 0:2, offs[0]:512],
                                     AFT.Exp, scale=SCALE)
            else:
                for t in range(2):
                    nc.scalar.activation(ex[:, t, offs[t]:512],
                                         psc[:, t, offs[t]:512],
                                         AFT.Exp, scale=SCALE)
            # triangular mask on the diagonal 128-block
            for t in range(2):
                j = jj + t
                dd = j - 4 * q
                if dd >= 0:
                    nc.vector.tensor_mul(
                        ex[:, t, dd * P:(dd + 1) * P],
                        ex[:, t, dd * P:(dd + 1) * P], msk_sb[:])
            state[(h, q, jj)] = ex

        def acc_step(i):
            h, q, jj, jmax = steps[i]
            ex = state.pop((h, q, jj))
            key = (h, q)
            if jj == 0:
                pden = pap.tile([P, 512], F32, name=f"pden{h}{q}",
                                tag="pden", bufs=2)
                pov = pap.tile([P, 512], F32, name=f"pov{h}{q}",
                               tag="pov", bufs=2)
                state[key] = [pden, pov, None]
            pden, pov, gacc = state[key]
            for t in range(2):
                j = jj + t
                dd = j - 4 * q
                off = dd * P if dd > 0 else 0
                nc.tensor.matmul(pov[:, off:512],
                                 vN[:, j, h * HD:(h + 1) * HD],
                                 ex[:, t, off:512],
                                 start=(j == 0), stop=(j == jmax - 1))
            # denominator: the ones matmul is linear in ex and shares its
            # stationary, so 4 full-width ex chunks are pre-summed on
            # VectorE and fed to ONE matmul. Diagonal (partial-N) chunks
            # and q==0 go straight to per-chunk partial matmuls.
            if q == 0 or jj >= jmax - 4:
                for t in range(2):
                    j = jj + t
                    dd = j - 4 * q
                    off = dd * P if dd > 0 else 0
                    nc.tensor.matmul(pden[:, off:512], ones_mat[:],
                                     ex[:, t, off:512],
                                     start=(j == 0), stop=(j == jmax - 1))
            elif jj % 4 == 0:
                gacc = apool.tile([P, 512], BF16, name=f"ga{h}{q}{jj}",
                                  tag="gacc")
                state[key][2] = gacc
                nc.vector.tensor_add(gacc[:], ex[:, 0, :], ex[:, 1, :])
            else:
                nc.vector.tensor_add(gacc[:], gacc[:], ex[:, 0, :])
                nc.vector.tensor_add(gacc[:], gacc[:], ex[:, 1, :])
                nc.tensor.matmul(pden[:], ones_mat[:], gacc[:],
                                 start=(jj == 2), stop=False)
            if jj == jmax - 2:
                del state[key]
                rbc = apool.tile([P, 512], F32, name=f"rbc{h}{q}", tag="rbc")
                nc.vector.reciprocal_approx_fast(rbc[:], pden[:])
                nc.vector.tensor_mul(
                    oT[:, h, q * 512:(q + 1) * 512],
                    pov[:], rbc[:])

        def oproj_burst(qc):
            # o_proj for this q-chunk: weight-stationary over 16 dt blocks,
            # PSUM shared with the score tiles ("psc" tag), evictions on
            # VectorE (ScalarE is saturated by exp), bf16 stores
            qsl = slice(qc * 512, (qc + 1) * 512)
            for dt2 in range(8):
                po = pap.tile([P, 2, 512], F32, name=f"po{qc}_{dt2}",
                              tag="psc", bufs=2)
                for t in range(2):
                    dt = dt2 * 2 + t
                    for h in range(H):
                        nc.tensor.matmul(po[:, t, :],
                                         wo_sb[:, h, dt * P:(dt + 1) * P],
                                         oT[:, h, qsl],
                                         start=(h == 0), stop=(h == H - 1))
                ot = oevp.tile([P, 2, 512], BF16, name=f"ot{qc}_{dt2}",
                               tag="ot")
                if qc == NQ - 1 and dt2 % 2 == 1:
                    # last q-chunk: exp is finished, ScalarE is free to
                    # share eviction duty and shorten the tail
                    nc.scalar.activation(ot[:], po[:], AFT.Copy)
                else:
                    nc.vector.tensor_copy(ot[:], po[:])
                # stores alternate between the Sync and Scalar DMA queues
                eng = nc.sync if dt2 % 2 == 0 else nc.scalar
                for t in range(2):
                    dt = dt2 * 2 + t
                    eng.dma_start(out[dt * P:(dt + 1) * P, qsl], ot[:, t, :])

        def acc_and_maybe_burst(i):
            acc_step(i)
            h, q, jj, jmax = steps[i]
            if h == H - 1 and jj == jmax - 2:
                oproj_burst(q)

        # software pipeline: scores two steps ahead of accumulation
        n = len(steps)
        for i in range(n):
            score_step(i)
            if i >= 2:
                acc_and_maybe_burst(i - 2)
        acc_and_maybe_burst(n - 2)
        acc_and_maybe_burst(n - 1)


def _body(tc, xT, wqT, wkT, wvT, woT, cosT, sinT, mskT, out):
    nc = tc.nc
    # long-lived slabs; left stack for qkv, right for attention-era tensors
    p_qk = tc.alloc_tile_pool(name="p_qk", bufs=1, side="left")
    qT = p_qk.tile([P, H, S], BF16)   # [hd, head, seq]
    kT = p_qk.tile([P, H, S], BF16)
    p_v = tc.alloc_tile_pool(name="p_v", bufs=1, side="left")
    vN = p_v.tile([P, NJ, F], BF16)   # [:, j, :] = v[j*128:(j+1)*128, :]

    _proj_phase(tc, xT, wqT, wkT, wvT, cosT, sinT, qT, kT, vN)

    p_oT = tc.alloc_tile_pool(name="p_oT", bufs=1, side="right")
    oT = p_oT.tile([P, H, S], BF16)   # attention output, transposed
    p_wo = tc.alloc_tile_pool(name="p_wo", bufs=1, side="right")
    wo_sb = p_wo.tile([P, H, D], BF16)

    _attn_phase(tc, mskT, woT, qT, kT, vN, oT, wo_sb, out)

    p_v.release()
    p_qk.release()
    p_wo.release()
    p_oT.release()


def build_nc():
    nc = bacc.Bacc("TRN2", target_bir_lowering=False, debug=False,
                   enable_asserts=True, num_devices=8)
    xT = nc.dram_tensor("xT", [D, S], BF16, kind="ExternalInput").ap()
    wqT = nc.dram_tensor("wqT", [D, F], BF16, kind="ExternalInput").ap()
    wkT = nc.dram_tensor("wkT", [D, F], BF16, kind="ExternalInput").ap()
    wvT = nc.dram_tensor("wvT", [D, F], BF16, kind="ExternalInput").ap()
    woT = nc.dram_tensor("woT", [F, D], BF16, kind="ExternalInput").ap()
    cosT = nc.dram_tensor("cosT", [P, S], BF16, kind="ExternalInput").ap()
    sinT = nc.dram_tensor("sinT", [P, S], BF16, kind="ExternalInput").ap()
    mskT = nc.dram_tensor("mskT", [P, P], BF16, kind="ExternalInput").ap()
    out = nc.dram_tensor("out", [S, D], BF16, kind="ExternalOutput").ap()

    with tile.TileContext(nc) as tc:
        _body(tc, xT, wqT, wkT, wvT, woT, cosT, sinT, mskT, out)
    nc.compile()
    return nc


_CACHE = {}


def _get_nc():
    if "nc" not in _CACHE:
        _CACHE["nc"] = build_nc()
    return _CACHE["nc"]


def _rope_tables():
    hd = HD
    inv = 1.0 / (10000.0 ** (np.arange(0, hd, 2, dtype=np.float32) / np.float32(hd)))
    t = np.arange(S, dtype=np.float32)
    freqs = np.outer(t, inv)                      # [S, 64]
    emb = np.concatenate([freqs, freqs], axis=-1)  # [S, 128]
    cosT = np.cos(emb).T.astype(np.float32).copy()
    sinT = np.sin(emb).T.astype(np.float32).copy()
    sinT[0:64, :] *= -1.0  # sign of rotate_half baked into the table
    return np.ascontiguousarray(cosT), np.ascontiguousarray(sinT)


def _diag_masks():
    kp = np.arange(P)[:, None]
    qf = np.arange(P)[None, :]
    return np.ascontiguousarray((kp <= qf).astype(np.float32))


def _in_maps(x, Wq, Wk, Wv, Wo):
    cosT, sinT = _rope_tables()
    msk = _diag_masks()
    BF = ml_dtypes.bfloat16
    maps = []
    for c in range(8):
        b, g = c // 4, c % 4
        fs = slice(g * F, (g + 1) * F)
        maps.append({
            "xT": np.ascontiguousarray(x[b].T).astype(BF),
            "wqT": np.ascontiguousarray(Wq[fs, :].T).astype(BF),
            "wkT": np.ascontiguousarray(Wk[fs, :].T).astype(BF),
            "wvT": np.ascontiguousarray(Wv[fs, :].T).astype(BF),
            "woT": np.ascontiguousarray(Wo[:, fs].T).astype(BF),
            "cosT": cosT.astype(BF),
            "sinT": sinT.astype(BF),
            "mskT": msk.astype(BF),
        })
    return maps


def run(x, Wq, Wk, Wv, Wo, trace=False, **spmd_kwargs):
    """Run on 8 cores; returns (full_output, BassKernelResults)."""
    x = np.asarray(x, np.float32)
    Wq = np.asarray(Wq, np.float32)
    Wk = np.asarray(Wk, np.float32)
    Wv = np.asarray(Wv, np.float32)
    Wo = np.asarray(Wo, np.float32)
    nc = _get_nc()
    maps = _in_maps(x, Wq, Wk, Wv, Wo)
    res = bass_utils.run_bass_kernel_spmd(nc, maps, core_ids=list(range(8)),
                                          trace=trace, **spmd_kwargs)
    outs = [res.results[c]["out"].astype(np.float32) for c in range(8)]
    full = np.empty((2, S, D), np.float32)
    for b in range(2):
        # each core returns its o_proj partial TRANSPOSED ([D_out, S])
        acc = outs[4 * b] + outs[4 * b + 1] + outs[4 * b + 2] + outs[4 * b + 3]
        full[b] = acc.T
    return full, res


def kernel(x, Wq, Wk, Wv, Wo):
    full, _ = run(x, Wq, Wk, Wv, Wo)
    return full


# revision 30
# speedup vs baseline: 1.0127x; 1.0008x over previous
"""Multi-head attention (RoPE, causal, fp32) on 8 Trainium2 NeuronCores.

Problem: B=2, S=2048, D=2048, H=16 heads (hd=128).
Sharding: DP=2 (batch) x TP=4 (head groups of 4 heads). Core c handles
batch c//4, head group c%4. Each core computes q/k/v projections for its
512 features, RoPE, causal attention, and a partial o_proj against its
512 columns of Wo. The host sums the 4 partial o_proj outputs per batch.

Layout strategy (per core), all compute in bf16 with fp32 PSUM
accumulation (rel-err budget 2e-2; measured ~6.5e-3):
  - All inputs host-cast to bf16: halves HBM traffic and enables FWL
    fast weight loads. Per-chunk DMAs are spread across the Sync,
    Scalar and GpSimd engine queues — each dma_start lands on its own
    hardware ring, so many small transfers run in parallel and the
    ~0.6us/DMA issue rate never starves the PE.
  - Single fused pass over x: per 512-seq chunk, three j-loops (q all
    heads, k all heads, v) each accumulate a 4-bank PSUM tile; the
    "acc" tag rotates through 2 slots so evictions (ScalarE) always
    overlap the next loop's matmuls. x is read from HBM exactly once.
  - RoPE applied on the bf16 slabs after eviction (rowswap via
    SBUF->SBUF DMA on the GpSimd queue, sign baked into the host sin
    table), on VectorE.
  - Attention entirely in transposed space: scoresT[k, q] tiles,
    lhsT=kT slice, rhs=qT chunk, exp fused into the PSUM eviction on
    ScalarE. One global software pipeline across ALL (head, q-chunk)
    pairs — score j-pair steps run two steps ahead of the denominator/
    attn@V accumulation, with per-j-pair ex tiles so the pipeline never
    drains at pair boundaries.
  - Causal: only j <= q k-tiles computed; diagonal k-tiles compute only
    the alive (q >= k) column range (partial-N matmuls for score, den,
    attn@V and the exp eviction), one shared [128,128] triangular mask
    multiplied on the edge block.
  - Softmax denominator via all-ones [128,128] stationary matmul (k-sum
    pre-broadcast across partitions); 1/denom via one DVE
    reciprocal_approx_fast; normalization folded into the attn@V
    eviction.
  - Attention is q-chunk-major and the o_proj for each q-chunk runs as
    a burst right after its last head finishes, sharing PSUM with the
    score tiles: output stores start ~60us early and the kernel tail is
    only the last chunk's drain. Partials stored TRANSPOSED ([D_out,S])
    in bf16; host sums the 4 per-batch partials in fp32.

Known-dead ends (measured slower or incorrect on HW): col-tiled 32-wide
denominator strips (tile-mode-switch drains + nondeterministic results
next to fp32r matmuls), GpSimd partition_all_reduce denominators
(~3.7us per [128,512] — serializes each q-block), whole-tensor batched
DMAs (one ring per dma_start caps at ~35GB/s).
"""

import sys

for _p in ("/opt/trn_rl_repo",):
    if _p not in sys.path:
        sys.path.insert(0, _p)

import ml_dtypes
import numpy as np

import concourse.bass as bass
import concourse.mybir as mybir
import concourse.tile as tile
from concourse import bacc, bass_utils


# NOTE: the baseline's --enable-ldw-opt=true patch is incompatible with
# bf16 LDWEIGHTS (walrus rejects FWL loads under ldw-opt), so it is not
# used here. LDWEIGHTS issue is hidden under matmul streaming via the
# dual SBUF read ports, so the elision is not needed.

P = 128          # partitions / head dim
S = 2048         # sequence length
D = 2048         # model dim
F = 512          # features per core (4 heads)
H = 4            # heads per core
HD = 128         # head dim
NJ = D // P      # 16 contraction chunks of 128
NQ = S // 512    # 4 query chunks of 512
SCALE = 1.0 / float(np.sqrt(HD))

F32 = mybir.dt.float32
BF16 = mybir.dt.bfloat16
AFT = mybir.ActivationFunctionType


def _proj_phase(tc, xT, wqT, wkT, wvT, cosT, sinT, qT, kT, vN):
    """Fused q/k/v projections + RoPE: one pass over x.

    Per 512-seq chunk, three j-loops (q all heads, k all heads, v) each
    accumulate a 4-bank PSUM tile; the "acc" tag rotates through 2 slots
    (8 banks) so evictions always overlap the next loop's matmuls. At
    s==0 the input DMA is staggered so the q-loop only carries the
    tensors it needs (x chunk + Wq + half of Wk): the PE is compute-
    paced from the first matmul on.
    """
    nc = tc.nc
    # dram views with the j-chunk (128-row blocks) split out: one strided
    # DMA can then load a whole tensor (the Sync engine's ~0.6us per-DMA
    # issue rate is the s==0 bottleneck, not HBM bandwidth)
    xR = xT.rearrange("(j p) s -> p j s", p=P)
    wqR = wqT.rearrange("(j p) f -> p j f", p=P)
    wkR = wkT.rearrange("(j p) f -> p j f", p=P)
    wvR = wvT.rearrange("(j p) f -> p j f", p=P)
    with tc.tile_pool(name="cs", bufs=1) as cspool, \
         tc.tile_pool(name="w", bufs=1) as wpool, \
         tc.tile_pool(name="xs", bufs=2) as xspool, \
         tc.tile_pool(name="rope", bufs=4) as rpool, \
         tc.tile_pool(name="pp", bufs=1, space="PSUM") as pp:
        cos_sb = cspool.tile([P, S], BF16)
        sin_sb = cspool.tile([P, S], BF16)
        wq_sb = wpool.tile([P, NJ, F], BF16)
        wk_sb = wpool.tile([P, NJ, F], BF16)
        wv_sb = wpool.tile([P, NJ, F], BF16)

        xs_tiles = {}

        def load_xs(s):
            xs = xspool.tile([P, NJ, 512], BF16, name=f"xs{s}", tag="xs")
            xs_tiles[s] = xs
            return xs

        xs0 = load_xs(0)
        # per-j DMAs (each dma_start lands on its own hardware ring, so
        # many small transfers run in parallel) split across the three
        # DMA-capable engine queues to parallelize the ~0.6us-per-DMA
        # issue cost: Sync=x, Scalar=wq then wv, GpSimd=cos/sin then wk
        for j in range(NJ):
            xeng = nc.sync if j % 2 == 0 else nc.gpsimd
            xeng.dma_start(xs0[:, j, :], xR[:, j, 0:512])
            # j==0 wq rides the Sync queue: the Scalar queue's first DMA
            # sits behind the ~1.3us ACT_TABLE_LOAD preamble
            weng = nc.sync if j == 0 else nc.scalar
            weng.dma_start(wq_sb[:, j, :], wqR[:, j, :])
        nc.gpsimd.dma_start(cos_sb[:], cosT)
        nc.gpsimd.dma_start(sin_sb[:], sinT)
        for j in range(NJ):
            nc.gpsimd.dma_start(wk_sb[:, j, :], wkR[:, j, :])
            nc.scalar.dma_start(wv_sb[:, j, :], wvR[:, j, :])

        def rope(dst, sl, sh, h):
            # dst = dst*cos + rowswap(dst)*sin, in place on the slab (bf16)
            rt = rpool.tile([P, 512], BF16, name=f"rt{sh}_{h}", tag="rt")
            nc.gpsimd.dma_start(rt[0:64, :], dst[64:128, :])
            nc.gpsimd.dma_start(rt[64:128, :], dst[0:64, :])
            nc.vector.tensor_mul(rt[:], rt[:], sin_sb[:, sl])
            nc.vector.tensor_mul(dst, dst, cos_sb[:, sl])
            nc.vector.tensor_add(dst, dst, rt[:])

        for s in range(NQ):
            sl = slice(s * 512, (s + 1) * 512)
            xs = xs_tiles[s]
            # ---- q loop (all 4 heads) ----
            pq = pp.tile([P, H, 512], F32, name=f"pq{s}", tag="acc", bufs=2)
            for j in range(NJ):
                for h in range(H):
                    nc.tensor.matmul(pq[:, h, :],
                                     wq_sb[:, j, h * HD:(h + 1) * HD],
                                     xs[:, j, :],
                                     start=(j == 0), stop=(j == NJ - 1))

            nc.scalar.activation(qT[:, :, sl], pq[:], AFT.Copy)
            for h in range(H):
                rope(qT[:, h, sl], sl, s, f"q{h}")
            # ---- k loop (all 4 heads) ----
            pk = pp.tile([P, H, 512], F32, name=f"pk{s}", tag="acc", bufs=2)
            for j in range(NJ):
                for h in range(H):
                    nc.tensor.matmul(pk[:, h, :],
                                     wk_sb[:, j, h * HD:(h + 1) * HD],
                                     xs[:, j, :],
                                     start=(j == 0), stop=(j == NJ - 1))

            nc.scalar.activation(kT[:, :, sl], pk[:], AFT.Copy)
            for h in range(H):
                rope(kT[:, h, sl], sl, s, f"k{h}")
            # ---- v loop (x chunks become the stationaries) ----
            pv = pp.tile([P, H, 512], F32, name=f"pv{s}", tag="acc", bufs=2)
            for j in range(NJ):
                for st in range(4):
                    nc.tensor.matmul(pv[:, st, :],
                                     xs[:, j, st * P:(st + 1) * P],
                                     wv_sb[:, j, :],
                                     start=(j == 0), stop=(j == NJ - 1))
                if s < NQ - 1:
                    xsn = load_xs(s + 1) if j == 0 else xs_tiles[s + 1]
                    nc.sync.dma_start(xsn[:, j, :],
                                      xR[:, j, (s + 1) * 512:(s + 2) * 512])
            nc.scalar.activation(vN[:, 4 * s:4 * s + 4, :], pv[:], AFT.Copy)


def _attn_phase(tc, mskT, woT, qT, kT, vN, oT, wo_sb, out):
    """Causal attention, one global software pipeline over (h, q, j-pair)."""
    nc = tc.nc
    with tc.tile_pool(name="amsk", bufs=1, side="right") as mpool, \
         tc.tile_pool(name="exp", bufs=5, side="right") as epool, \
         tc.tile_pool(name="attsb", bufs=2, side="right") as apool, \
         tc.tile_pool(name="oev", bufs=3, side="right") as oevp, \
         tc.tile_pool(name="pa", bufs=1, space="PSUM") as pap:
        msk_sb = mpool.tile([P, P], BF16)
        nc.sync.dma_start(msk_sb[:], mskT)
        # all-ones [128,128] stationary: the denominator matmul then yields
        # the k-sum already broadcast across all 128 partitions of PSUM.
        ones_tmp = mpool.tile([P, P], F32)
        nc.vector.memset(ones_tmp[:], 1.0)
        ones_mat = mpool.tile([P, P], BF16)
        nc.vector.tensor_copy(ones_mat[:], ones_tmp[:])
        # prefetch Wo during attention compute (Scalar queue, per head)
        woR = woT.rearrange("(h p) d -> p h d", p=P)
        for h in range(H):
            nc.scalar.dma_start(wo_sb[:, h, :], woR[:, h, :])

        # ---- flattened list of score/acc steps, q-major so each
        # q-chunk's o_proj burst can run as soon as all heads finish ----
        steps = []
        for q in range(NQ):
            for h in range(H):
                jmax = 4 * (q + 1)
                for jj in range(0, jmax, 2):
                    steps.append((h, q, jj, jmax))
        state = {}  # (h,q) -> dict with psum tiles + ex tiles per step

        def score_step(i):
            h, q, jj, jmax = steps[i]
            qh = qT[:, h, :]
            kh = kT[:, h, :]
            ex = epool.tile([P, 2, 512], BF16, name=f"ex{h}_{q}_{jj}", tag="ex")
            psc = pap.tile([P, 2, 512], F32, name=f"psc{h}{q}{jj}",
                           tag="psc", bufs=2)
            offs = []
            for t in range(2):
                j = jj + t
                dd = j - 4 * q
                off = dd * P if dd > 0 else 0
                offs.append(off)
                nc.tensor.matmul(psc[:, t, off:512],
                                 kh[:, j * P:(j + 1) * P],
                                 qh[:, q * 512 + off:(q + 1) * 512],
                                 start=True, stop=True)
            if offs[0] == offs[1]:
                # same width: one fused exp eviction for the pair
                nc.scalar.activation(ex[:, 0:2, offs[0]:512],
                                     psc[:, 0:2, offs[0]:512],
                                     AFT.Exp, scale=SCALE)
            else:
                for t in range(2):
                    nc.scalar.activation(ex[:, t, offs[t]:512],
                                         psc[:, t, offs[t]:512],
                                         AFT.Exp, scale=SCALE)
            # triangular mask on the diagonal 128-block
            for t in range(2):
                j = jj + t
                dd = j - 4 * q
                if dd >= 0:
                    nc.vector.tensor_mul(
                        ex[:, t, dd * P:(dd + 1) * P],
                        ex[:, t, dd * P:(dd + 1) * P], msk_sb[:])
            # den-group pre-sums issued at score time so the grouped
            # denominator matmul (two steps later) never waits on VectorE
            if q > 0 and jj < jmax - 4:
                if jj % 4 == 0:
                    gacc = epool.tile([P, 512], BF16, name=f"ga{h}{q}{jj}",
                                      tag="gacc", bufs=2)
                    state[(h, q, 'g', jj // 4)] = gacc
                    nc.vector.tensor_add(gacc[:], ex[:, 0, :], ex[:, 1, :])
                else:
                    gacc = state[(h, q, 'g', jj // 4)]
                    nc.vector.tensor_add(gacc[:], gacc[:], ex[:, 0, :])
                    nc.vector.tensor_add(gacc[:], gacc[:], ex[:, 1, :])
            state[(h, q, jj)] = ex

        def acc_step(i):
            h, q, jj, jmax = steps[i]
            ex = state.pop((h, q, jj))
            key = (h, q)
            if jj == 0:
                pden = pap.tile([P, 512], F32, name=f"pden{h}{q}",
                                tag="pden", bufs=2)
                pov = pap.tile([P, 512], F32, name=f"pov{h}{q}",
                               tag="pov", bufs=2)
                state[key] = (pden, pov)
            pden, pov = state[key]
            for t in range(2):
                j = jj + t
                dd = j - 4 * q
                off = dd * P if dd > 0 else 0
                nc.tensor.matmul(pov[:, off:512],
                                 vN[:, j, h * HD:(h + 1) * HD],
                                 ex[:, t, off:512],
                                 start=(j == 0), stop=(j == jmax - 1))
            # denominator: the ones matmul is linear in ex and shares its
            # stationary, so 4 full-width ex chunks were pre-summed on
            # VectorE at score time and feed ONE matmul here. Diagonal
            # (partial-N) chunks and q==0 go straight to per-chunk
            # partial matmuls.
            if q == 0 or jj >= jmax - 4:
                for t in range(2):
                    j = jj + t
                    dd = j - 4 * q
                    off = dd * P if dd > 0 else 0
                    nc.tensor.matmul(pden[:, off:512], ones_mat[:],
                                     ex[:, t, off:512],
                                     start=(j == 0), stop=(j == jmax - 1))
            elif jj % 4 == 2:
                gacc = state.pop((h, q, 'g', jj // 4))
                nc.tensor.matmul(pden[:], ones_mat[:], gacc[:],
                                 start=(jj == 2), stop=False)
            if jj == jmax - 2:
                del state[key]
                rbc = apool.tile([P, 512], F32, name=f"rbc{h}{q}", tag="rbc")
                nc.vector.reciprocal_approx_fast(rbc[:], pden[:])
                nc.vector.tensor_mul(
                    oT[:, h, q * 512:(q + 1) * 512],
                    pov[:], rbc[:])

        def oproj_burst(qc):
            # o_proj for this q-chunk: weight-stationary over 16 dt blocks,
            # PSUM shared with the score tiles ("psc" tag), evictions on
            # VectorE (ScalarE is saturated by exp), bf16 stores
            qsl = slice(qc * 512, (qc + 1) * 512)
            for dt2 in range(8):
                po = pap.tile([P, 2, 512], F32, name=f"po{qc}_{dt2}",
                              tag="psc", bufs=2)
                for t in range(2):
                    dt = dt2 * 2 + t
                    for h in range(H):
                        nc.tensor.matmul(po[:, t, :],
                                         wo_sb[:, h, dt * P:(dt + 1) * P],
                                         oT[:, h, qsl],
                                         start=(h == 0), stop=(h == H - 1))
                ot = oevp.tile([P, 2, 512], BF16, name=f"ot{qc}_{dt2}",
                               tag="ot")
                if qc == NQ - 1 and dt2 % 2 == 1:
                    # last q-chunk: exp is finished, ScalarE is free to
                    # share eviction duty and shorten the tail
                    nc.scalar.activation(ot[:], po[:], AFT.Copy)
                else:
                    nc.vector.tensor_copy(ot[:], po[:])
                # stores alternate between the Sync and Scalar DMA queues
                eng = nc.sync if dt2 % 2 == 0 else nc.scalar
                for t in range(2):
                    dt = dt2 * 2 + t
                    eng.dma_start(out[dt * P:(dt + 1) * P, qsl], ot[:, t, :])

        def acc_and_maybe_burst(i):
            acc_step(i)
            h, q, jj, jmax = steps[i]
            if h == H - 1 and jj == jmax - 2:
                oproj_burst(q)

        # software pipeline: scores two steps ahead of accumulation
        n = len(steps)
        for i in range(n):
            score_step(i)
            if i >= 2:
                acc_and_maybe_burst(i - 2)
        acc_and_maybe_burst(n - 2)
        acc_and_maybe_burst(n - 1)


def _body(tc, xT, wqT, wkT, wvT, woT, cosT, sinT, mskT, out):
    nc = tc.nc
    # long-lived slabs; left stack for qkv, right for attention-era tensors
    p_qk = tc.alloc_tile_pool(name="p_qk", bufs=1, side="left")
    qT = p_qk.tile([P, H, S], BF16)   # [hd, head, seq]
    kT = p_qk.tile([P, H, S], BF16)
    p_v = tc.alloc_tile_pool(name="p_v", bufs=1, side="left")
    vN = p_v.tile([P, NJ, F], BF16)   # [:, j, :] = v[j*128:(j+1)*128, :]

    _proj_phase(tc, xT, wqT, wkT, wvT, cosT, sinT, qT, kT, vN)

    p_oT = tc.alloc_tile_pool(name="p_oT", bufs=1, side="right")
    oT = p_oT.tile([P, H, S], BF16)   # attention output, transposed
    p_wo = tc.alloc_tile_pool(name="p_wo", bufs=1, side="right")
    wo_sb = p_wo.tile([P, H, D], BF16)

    _attn_phase(tc, mskT, woT, qT, kT, vN, oT, wo_sb, out)

    p_v.release()
    p_qk.release()
    p_wo.release()
    p_oT.release()


def build_nc():
    nc = bacc.Bacc("TRN2", target_bir_lowering=False, debug=False,
                   enable_asserts=True, num_devices=8)
    xT = nc.dram_tensor("xT", [D, S], BF16, kind="ExternalInput").ap()
    wqT = nc.dram_tensor("wqT", [D, F], BF16, kind="ExternalInput").ap()
    wkT = nc.dram_tensor("wkT", [D, F], BF16, kind="ExternalInput").ap()
    wvT = nc.dram_tensor("wvT", [D, F], BF16, kind="ExternalInput").ap()
    woT = nc.dram_tensor("woT", [F, D], BF16, kind="ExternalInput").ap()
    cosT = nc.dram_tensor("cosT", [P, S], BF16, kind="ExternalInput").ap()
    sinT = nc.dram_tensor("sinT", [P, S], BF16, kind="ExternalInput").ap()
    mskT = nc.dram_tensor("mskT", [P, P], BF16, kind="ExternalInput").ap()
    out = nc.dram_tensor("out", [S, D], BF16, kind="ExternalOutput").ap()

    with tile.TileContext(nc) as tc:
        _body(tc, xT, wqT, wkT, wvT, woT, cosT, sinT, mskT, out)
    nc.compile()
    return nc


_CACHE = {}


def _get_nc():
    if "nc" not in _CACHE:
        _CACHE["nc"] = build_nc()
    return _CACHE["nc"]


def _rope_tables():
    hd = HD
    inv = 1.0 / (10000.0 ** (np.arange(0, hd, 2, dtype=np.float32) / np.float32(hd)))
    t = np.arange(S, dtype=np.float32)
    freqs = np.outer(t, inv)                      # [S, 64]
    emb = np.concatenate([freqs, freqs], axis=-1)  # [S, 128]
    cosT = np.cos(emb).T.astype(np.float32).copy()
    sinT = np.sin(emb).T.astype(np.float32).copy()
    sinT[0:64, :] *= -1.0  # sign of rotate_half baked into the table
    return np.ascontiguousarray(cosT), np.ascontiguousarray(sinT)


def _diag_masks():
    kp = np.arange(P)[:, None]
    qf = np.arange(P)[None, :]
    return np.ascontiguousarray((kp <= qf).astype(np.float32))


def _in_maps(x, Wq, Wk, Wv, Wo):
    cosT, sinT = _rope_tables()
    msk = _diag_masks()
    BF = ml_dtypes.bfloat16
    maps = []
    for c in range(8):
        b, g = c // 4, c % 4
        fs = slice(g * F, (g + 1) * F)
        maps.append({
            "xT": np.ascontiguousarray(x[b].T).astype(BF),
            "wqT": np.ascontiguousarray(Wq[fs, :].T).astype(BF),
            "wkT": np.ascontiguousarray(Wk[fs, :].T).astype(BF),
            "wvT": np.ascontiguousarray(Wv[fs, :].T).astype(BF),
            "woT": np.ascontiguousarray(Wo[:, fs].T).astype(BF),
            "cosT": cosT.astype(BF),
            "sinT": sinT.astype(BF),
            "mskT": msk.astype(BF),
        })
    return maps


def run(x, Wq, Wk, Wv, Wo, trace=False, **spmd_kwargs):
    """Run on 8 cores; returns (full_output, BassKernelResults)."""
    x = np.asarray(x, np.float32)
    Wq = np.asarray(Wq, np.float32)
    Wk = np.asarray(Wk, np.float32)
    Wv = np.asarray(Wv, np.float32)
    Wo = np.asarray(Wo, np.float32)
    nc = _get_nc()
    maps = _in_maps(x, Wq, Wk, Wv, Wo)
    res = bass_utils.run_bass_kernel_spmd(nc, maps, core_ids=list(range(8)),
                                          trace=trace, **spmd_kwargs)
    outs = [res.results[c]["out"].astype(np.float32) for c in range(8)]
    full = np.empty((2, S, D), np.float32)
    for b in range(2):
        # each core returns its o_proj partial TRANSPOSED ([D_out, S])
        acc = outs[4 * b] + outs[4 * b + 1] + outs[4 * b + 2] + outs[4 * b + 3]
        full[b] = acc.T
    return full, res


def kernel(x, Wq, Wk, Wv, Wo):
    full, _ = run(x, Wq, Wk, Wv, Wo)
    return full


# revision 31
# speedup vs baseline: 1.0230x; 1.0102x over previous
"""Multi-head attention (RoPE, causal, fp32) on 8 Trainium2 NeuronCores.

Problem: B=2, S=2048, D=2048, H=16 heads (hd=128).
Sharding: DP=2 (batch) x TP=4 (head groups of 4 heads). Core c handles
batch c//4, head group c%4. Each core computes q/k/v projections for its
512 features, RoPE, causal attention, and a partial o_proj against its
512 columns of Wo. The host sums the 4 partial o_proj outputs per batch.

Layout strategy (per core), all compute in bf16 with fp32 PSUM
accumulation (rel-err budget 2e-2; measured ~6.5e-3):
  - All inputs host-cast to bf16: halves HBM traffic and enables FWL
    fast weight loads. Per-chunk DMAs are spread across the Sync,
    Scalar and GpSimd engine queues — each dma_start lands on its own
    hardware ring, so many small transfers run in parallel and the
    ~0.6us/DMA issue rate never starves the PE.
  - Single fused pass over x: per 512-seq chunk, three j-loops (q all
    heads, k all heads, v) each accumulate a 4-bank PSUM tile; the
    "acc" tag rotates through 2 slots so evictions (ScalarE) always
    overlap the next loop's matmuls. x is read from HBM exactly once.
  - RoPE applied on the bf16 slabs after eviction (rowswap via
    SBUF->SBUF DMA on the GpSimd queue, sign baked into the host sin
    table), on VectorE.
  - Attention entirely in transposed space: scoresT[k, q] tiles,
    lhsT=kT slice, rhs=qT chunk, exp fused into the PSUM eviction on
    ScalarE. One global software pipeline across ALL (head, q-chunk)
    pairs — score j-pair steps run two steps ahead of the denominator/
    attn@V accumulation, with per-j-pair ex tiles so the pipeline never
    drains at pair boundaries.
  - Causal: only j <= q k-tiles computed; diagonal k-tiles compute only
    the alive (q >= k) column range (partial-N matmuls for score, den,
    attn@V and the exp eviction), one shared [128,128] triangular mask
    multiplied on the edge block.
  - Softmax denominator via all-ones [128,128] stationary matmul (k-sum
    pre-broadcast across partitions); 1/denom via one DVE
    reciprocal_approx_fast; normalization folded into the attn@V
    eviction.
  - Attention is q-chunk-major and the o_proj for each q-chunk runs as
    a burst right after its last head finishes, sharing PSUM with the
    score tiles: output stores start ~60us early and the kernel tail is
    only the last chunk's drain. Partials stored TRANSPOSED ([D_out,S])
    in bf16; host sums the 4 per-batch partials in fp32.

Known-dead ends (measured slower or incorrect on HW): col-tiled 32-wide
denominator strips (tile-mode-switch drains + nondeterministic results
next to fp32r matmuls), GpSimd partition_all_reduce denominators
(~3.7us per [128,512] — serializes each q-block), whole-tensor batched
DMAs (one ring per dma_start caps at ~35GB/s).
"""

import sys

for _p in ("/opt/trn_rl_repo",):
    if _p not in sys.path:
        sys.path.insert(0, _p)

import ml_dtypes
import numpy as np

import concourse.bass as bass
import concourse.mybir as mybir
import concourse.tile as tile
from concourse import bacc, bass_utils


# NOTE: the baseline's --enable-ldw-opt=true patch is incompatible with
# bf16 LDWEIGHTS (walrus rejects FWL loads under ldw-opt), so it is not
# used here. LDWEIGHTS issue is hidden under matmul streaming via the
# dual SBUF read ports, so the elision is not needed.

P = 128          # partitions / head dim
S = 2048         # sequence length
D = 2048         # model dim
F = 512          # features per core (4 heads)
H = 4            # heads per core
HD = 128         # head dim
NJ = D // P      # 16 contraction chunks of 128
NQ = S // 512    # 4 query chunks of 512
SCALE = 1.0 / float(np.sqrt(HD))

F32 = mybir.dt.float32
BF16 = mybir.dt.bfloat16
AFT = mybir.ActivationFunctionType


def _proj_phase(tc, xT, wqT, wkT, wvT, cosT, sinT, qT, kT, vN):
    """Fused q/k/v projections + RoPE: one pass over x.

    Per 512-seq chunk, three j-loops (q all heads, k all heads, v) each
    accumulate a 4-bank PSUM tile; the "acc" tag rotates through 2 slots
    (8 banks) so evictions always overlap the next loop's matmuls. At
    s==0 the input DMA is staggered so the q-loop only carries the
    tensors it needs (x chunk + Wq + half of Wk): the PE is compute-
    paced from the first matmul on.
    """
    nc = tc.nc
    # dram views with the j-chunk (128-row blocks) split out: one strided
    # DMA can then load a whole tensor (the Sync engine's ~0.6us per-DMA
    # issue rate is the s==0 bottleneck, not HBM bandwidth)
    xR = xT.rearrange("(j p) s -> p j s", p=P)
    wqR = wqT.rearrange("(j p) f -> p j f", p=P)
    wkR = wkT.rearrange("(j p) f -> p j f", p=P)
    wvR = wvT.rearrange("(j p) f -> p j f", p=P)
    with tc.tile_pool(name="cs", bufs=1) as cspool, \
         tc.tile_pool(name="w", bufs=1) as wpool, \
         tc.tile_pool(name="xs", bufs=2) as xspool, \
         tc.tile_pool(name="rope", bufs=4) as rpool, \
         tc.tile_pool(name="pp", bufs=1, space="PSUM") as pp:
        cos_sb = cspool.tile([P, S], BF16)
        sin_sb = cspool.tile([P, S], BF16)
        wq_sb = wpool.tile([P, NJ, F], BF16)
        wk_sb = wpool.tile([P, NJ, F], BF16)
        wv_sb = wpool.tile([P, NJ, F], BF16)

        xs_tiles = {}

        def load_xs(s):
            xs = xspool.tile([P, NJ, 512], BF16, name=f"xs{s}", tag="xs")
            xs_tiles[s] = xs
            return xs

        xs0 = load_xs(0)
        # per-j DMAs (each dma_start lands on its own hardware ring, so
        # many small transfers run in parallel) split across the three
        # DMA-capable engine queues to parallelize the ~0.6us-per-DMA
        # issue cost: Sync=x, Scalar=wq then wv, GpSimd=cos/sin then wk
        for j in range(NJ):
            xeng = nc.sync if j % 2 == 0 else nc.gpsimd
            xeng.dma_start(xs0[:, j, :], xR[:, j, 0:512])
            # j==0 wq rides the Sync queue: the Scalar queue's first DMA
            # sits behind the ~1.3us ACT_TABLE_LOAD preamble
            weng = nc.sync if j == 0 else nc.scalar
            weng.dma_start(wq_sb[:, j, :], wqR[:, j, :])
        nc.gpsimd.dma_start(cos_sb[:], cosT)
        nc.gpsimd.dma_start(sin_sb[:], sinT)
        for j in range(NJ):
            nc.gpsimd.dma_start(wk_sb[:, j, :], wkR[:, j, :])
            nc.scalar.dma_start(wv_sb[:, j, :], wvR[:, j, :])

        def rope(dst, sl, sh, h):
            # dst = dst*cos + rowswap(dst)*sin, in place on the slab (bf16)
            rt = rpool.tile([P, 512], BF16, name=f"rt{sh}_{h}", tag="rt")
            nc.gpsimd.dma_start(rt[0:64, :], dst[64:128, :])
            nc.gpsimd.dma_start(rt[64:128, :], dst[0:64, :])
            nc.vector.tensor_mul(rt[:], rt[:], sin_sb[:, sl])
            nc.vector.tensor_mul(dst, dst, cos_sb[:, sl])
            nc.vector.tensor_add(dst, dst, rt[:])

        for s in range(NQ):
            sl = slice(s * 512, (s + 1) * 512)
            xs = xs_tiles[s]
            # ---- q loop (all 4 heads) ----
            pq = pp.tile([P, H, 512], F32, name=f"pq{s}", tag="acc", bufs=2)
            for j in range(NJ):
                for h in range(H):
                    nc.tensor.matmul(pq[:, h, :],
                                     wq_sb[:, j, h * HD:(h + 1) * HD],
                                     xs[:, j, :],
                                     start=(j == 0), stop=(j == NJ - 1))

            nc.scalar.activation(qT[:, :, sl], pq[:], AFT.Copy)
            for h in range(H):
                rope(qT[:, h, sl], sl, s, f"q{h}")
            # ---- k loop (all 4 heads) ----
            pk = pp.tile([P, H, 512], F32, name=f"pk{s}", tag="acc", bufs=2)
            for j in range(NJ):
                for h in range(H):
                    nc.tensor.matmul(pk[:, h, :],
                                     wk_sb[:, j, h * HD:(h + 1) * HD],
                                     xs[:, j, :],
                                     start=(j == 0), stop=(j == NJ - 1))

            nc.scalar.activation(kT[:, :, sl], pk[:], AFT.Copy)
            for h in range(H):
                rope(kT[:, h, sl], sl, s, f"k{h}")
            # ---- v loop (x chunks become the stationaries) ----
            pv = pp.tile([P, H, 512], F32, name=f"pv{s}", tag="acc", bufs=2)
            for j in range(NJ):
                for st in range(4):
                    nc.tensor.matmul(pv[:, st, :],
                                     xs[:, j, st * P:(st + 1) * P],
                                     wv_sb[:, j, :],
                                     start=(j == 0), stop=(j == NJ - 1))
                if s < NQ - 1:
                    xsn = load_xs(s + 1) if j == 0 else xs_tiles[s + 1]
                    nc.sync.dma_start(xsn[:, j, :],
                                      xR[:, j, (s + 1) * 512:(s + 2) * 512])
            nc.scalar.activation(vN[:, 4 * s:4 * s + 4, :], pv[:], AFT.Copy)


def _attn_phase(tc, mskT, woT, qT, kT, vN, oT, wo_sb, out):
    """Causal attention, one global software pipeline over (h, q, j-pair)."""
    nc = tc.nc
    with tc.tile_pool(name="amsk", bufs=1, side="right") as mpool, \
         tc.tile_pool(name="exp", bufs=5, side="right") as epool, \
         tc.tile_pool(name="attsb", bufs=2, side="right") as apool, \
         tc.tile_pool(name="oev", bufs=3, side="right") as oevp, \
         tc.tile_pool(name="pa", bufs=1, space="PSUM") as pap:
        msk_sb = mpool.tile([P, P], BF16)
        nc.sync.dma_start(msk_sb[:], mskT)
        # all-ones [128,128] stationary: the denominator matmul then yields
        # the k-sum already broadcast across all 128 partitions of PSUM.
        ones_tmp = mpool.tile([P, P], F32)
        nc.vector.memset(ones_tmp[:], 1.0)
        ones_mat = mpool.tile([P, P], BF16)
        nc.vector.tensor_copy(ones_mat[:], ones_tmp[:])
        # prefetch Wo during attention compute (Scalar queue, per head)
        woR = woT.rearrange("(h p) d -> p h d", p=P)
        for h in range(H):
            nc.scalar.dma_start(wo_sb[:, h, :], woR[:, h, :])

        # ---- flattened list of score/acc steps, q-major so each
        # q-chunk's o_proj burst can run as soon as all heads finish ----
        steps = []
        for q in range(NQ):
            for h in range(H):
                jmax = 4 * (q + 1)
                for jj in range(0, jmax, 2):
                    steps.append((h, q, jj, jmax))
        state = {}  # (h,q) -> dict with psum tiles + ex tiles per step

        def score_step(i):
            h, q, jj, jmax = steps[i]
            qh = qT[:, h, :]
            kh = kT[:, h, :]
            ex = epool.tile([P, 2, 512], BF16, name=f"ex{h}_{q}_{jj}", tag="ex")
            psc = pap.tile([P, 2, 512], F32, name=f"psc{h}{q}{jj}",
                           tag="psc", bufs=2)
            offs = []
            for t in range(2):
                j = jj + t
                dd = j - 4 * q
                off = dd * P if dd > 0 else 0
                offs.append(off)
                nc.tensor.matmul(psc[:, t, off:512],
                                 kh[:, j * P:(j + 1) * P],
                                 qh[:, q * 512 + off:(q + 1) * 512],
                                 start=True, stop=True)
            if offs[0] == offs[1]:
                # same width: one fused exp eviction for the pair
                nc.scalar.activation(ex[:, 0:2, offs[0]:512],
                                     psc[:, 0:2, offs[0]:512],
                                     AFT.Exp, scale=SCALE)
            else:
                for t in range(2):
                    nc.scalar.activation(ex[:, t, offs[t]:512],
                                         psc[:, t, offs[t]:512],
                                         AFT.Exp, scale=SCALE)
            # triangular mask on the diagonal 128-block
            for t in range(2):
                j = jj + t
                dd = j - 4 * q
                if dd >= 0:
                    nc.vector.tensor_mul(
                        ex[:, t, dd * P:(dd + 1) * P],
                        ex[:, t, dd * P:(dd + 1) * P], msk_sb[:])
            # den-group pre-sums issued at score time so the grouped
            # denominator matmul (two steps later) never waits on VectorE
            if q > 0 and jj < jmax - 4:
                if jj % 4 == 0:
                    gacc = epool.tile([P, 512], BF16, name=f"ga{h}{q}{jj}",
                                      tag="gacc", bufs=2)
                    state[(h, q, 'g', jj // 4)] = gacc
                    nc.vector.tensor_add(gacc[:], ex[:, 0, :], ex[:, 1, :])
                else:
                    gacc = state[(h, q, 'g', jj // 4)]
                    nc.vector.tensor_add(gacc[:], gacc[:], ex[:, 0, :])
                    nc.vector.tensor_add(gacc[:], gacc[:], ex[:, 1, :])
            state[(h, q, jj)] = ex

        def acc_step(i):
            h, q, jj, jmax = steps[i]
            ex = state.pop((h, q, jj))
            key = (h, q)
            if jj == 0:
                pden = pap.tile([P, 512], F32, name=f"pden{h}{q}",
                                tag="pden", bufs=2)
                pov = pap.tile([P, 512], F32, name=f"pov{h}{q}",
                               tag="pov", bufs=2)
                state[key] = (pden, pov)
            pden, pov = state[key]
            for t in range(2):
                j = jj + t
                dd = j - 4 * q
                off = dd * P if dd > 0 else 0
                nc.tensor.matmul(pov[:, off:512],
                                 vN[:, j, h * HD:(h + 1) * HD],
                                 ex[:, t, off:512],
                                 start=(j == 0), stop=(j == jmax - 1))
            # denominator: the ones matmul is linear in ex and shares its
            # stationary, so 4 full-width ex chunks were pre-summed on
            # VectorE at score time and feed ONE matmul here. Diagonal
            # (partial-N) chunks and q==0 go straight to per-chunk
            # partial matmuls.
            if q == 0 or jj >= jmax - 4:
                for t in range(2):
                    j = jj + t
                    dd = j - 4 * q
                    off = dd * P if dd > 0 else 0
                    nc.tensor.matmul(pden[:, off:512], ones_mat[:],
                                     ex[:, t, off:512],
                                     start=(j == 0), stop=(j == jmax - 1))
            elif jj % 4 == 2:
                gacc = state.pop((h, q, 'g', jj // 4))
                nc.tensor.matmul(pden[:], ones_mat[:], gacc[:],
                                 start=(jj == 2), stop=False)
            if jj == jmax - 2:
                del state[key]
                rbc = apool.tile([P, 512], F32, name=f"rbc{h}{q}", tag="rbc")
                nc.vector.reciprocal_approx_fast(rbc[:], pden[:])
                nc.vector.tensor_mul(
                    oT[:, h, q * 512:(q + 1) * 512],
                    pov[:], rbc[:])

        def oproj_burst(qc):
            # o_proj for this q-chunk: weight-stationary over 16 dt blocks,
            # PSUM shared with the score tiles ("psc" tag), evictions on
            # VectorE (ScalarE is saturated by exp), bf16 stores
            qsl = slice(qc * 512, (qc + 1) * 512)
            # po tiles borrow the "pden" slots (denominators for this
            # block are already consumed), NOT the "psc" slots: the next
            # q-block's score pipeline keeps flowing through the burst
            # instead of restarting after it.
            for dt in range(D // P):
                po = pap.tile([P, 512], F32, name=f"po{qc}_{dt}",
                              tag="pden", bufs=2)
                for h in range(H):
                    nc.tensor.matmul(po[:],
                                     wo_sb[:, h, dt * P:(dt + 1) * P],
                                     oT[:, h, qsl],
                                     start=(h == 0), stop=(h == H - 1))
                ot = oevp.tile([P, 512], BF16, name=f"ot{qc}_{dt}",
                               tag="ot")
                if qc == NQ - 1 and dt % 2 == 1:
                    # last q-chunk: exp is finished, ScalarE is free to
                    # share eviction duty and shorten the tail
                    nc.scalar.activation(ot[:], po[:], AFT.Copy)
                else:
                    nc.vector.tensor_copy(ot[:], po[:])
                # stores alternate between the Sync and Scalar DMA queues
                eng = nc.sync if dt % 2 == 0 else nc.scalar
                eng.dma_start(out[dt * P:(dt + 1) * P, qsl], ot[:])

        def acc_and_maybe_burst(i):
            acc_step(i)
            h, q, jj, jmax = steps[i]
            if h == H - 1 and jj == jmax - 2:
                oproj_burst(q)

        # software pipeline: scores two steps ahead of accumulation
        n = len(steps)
        for i in range(n):
            score_step(i)
            if i >= 2:
                acc_and_maybe_burst(i - 2)
        acc_and_maybe_burst(n - 2)
        acc_and_maybe_burst(n - 1)


def _body(tc, xT, wqT, wkT, wvT, woT, cosT, sinT, mskT, out):
    nc = tc.nc
    # long-lived slabs; left stack for qkv, right for attention-era tensors
    p_qk = tc.alloc_tile_pool(name="p_qk", bufs=1, side="left")
    qT = p_qk.tile([P, H, S], BF16)   # [hd, head, seq]
    kT = p_qk.tile([P, H, S], BF16)
    p_v = tc.alloc_tile_pool(name="p_v", bufs=1, side="left")
    vN = p_v.tile([P, NJ, F], BF16)   # [:, j, :] = v[j*128:(j+1)*128, :]

    _proj_phase(tc, xT, wqT, wkT, wvT, cosT, sinT, qT, kT, vN)

    p_oT = tc.alloc_tile_pool(name="p_oT", bufs=1, side="right")
    oT = p_oT.tile([P, H, S], BF16)   # attention output, transposed
    p_wo = tc.alloc_tile_pool(name="p_wo", bufs=1, side="right")
    wo_sb = p_wo.tile([P, H, D], BF16)

    _attn_phase(tc, mskT, woT, qT, kT, vN, oT, wo_sb, out)

    p_v.release()
    p_qk.release()
    p_wo.release()
    p_oT.release()


def build_nc():
    nc = bacc.Bacc("TRN2", target_bir_lowering=False, debug=False,
                   enable_asserts=True, num_devices=8)
    xT = nc.dram_tensor("xT", [D, S], BF16, kind="ExternalInput").ap()
    wqT = nc.dram_tensor("wqT", [D, F], BF16, kind="ExternalInput").ap()
    wkT = nc.dram_tensor("wkT", [D, F], BF16, kind="ExternalInput").ap()
    wvT = nc.dram_tensor("wvT", [D, F], BF16, kind="ExternalInput").ap()
    woT = nc.dram_tensor("woT", [F, D], BF16, kind="ExternalInput").ap()
    cosT = nc.dram_tensor("cosT", [P, S], BF16, kind="ExternalInput").ap()
    sinT = nc.dram_tensor("sinT", [P, S], BF16, kind="ExternalInput").ap()
    mskT = nc.dram_tensor("mskT", [P, P], BF16, kind="ExternalInput").ap()
    out = nc.dram_tensor("out", [S, D], BF16, kind="ExternalOutput").ap()

    with tile.TileContext(nc) as tc:
        _body(tc, xT, wqT, wkT, wvT, woT, cosT, sinT, mskT, out)
    nc.compile()
    return nc


_CACHE = {}


def _get_nc():
    if "nc" not in _CACHE:
        _CACHE["nc"] = build_nc()
    return _CACHE["nc"]


def _rope_tables():
    hd = HD
    inv = 1.0 / (10000.0 ** (np.arange(0, hd, 2, dtype=np.float32) / np.float32(hd)))
    t = np.arange(S, dtype=np.float32)
    freqs = np.outer(t, inv)                      # [S, 64]
    emb = np.concatenate([freqs, freqs], axis=-1)  # [S, 128]
    cosT = np.cos(emb).T.astype(np.float32).copy()
    sinT = np.sin(emb).T.astype(np.float32).copy()
    sinT[0:64, :] *= -1.0  # sign of rotate_half baked into the table
    return np.ascontiguousarray(cosT), np.ascontiguousarray(sinT)


def _diag_masks():
    kp = np.arange(P)[:, None]
    qf = np.arange(P)[None, :]
    return np.ascontiguousarray((kp <= qf).astype(np.float32))


def _in_maps(x, Wq, Wk, Wv, Wo):
    cosT, sinT = _rope_tables()
    msk = _diag_masks()
    BF = ml_dtypes.bfloat16
    maps = []
    for c in range(8):
        b, g = c // 4, c % 4
        fs = slice(g * F, (g + 1) * F)
        maps.append({
            "xT": np.ascontiguousarray(x[b].T).astype(BF),
            "wqT": np.ascontiguousarray(Wq[fs, :].T).astype(BF),
            "wkT": np.ascontiguousarray(Wk[fs, :].T).astype(BF),
            "wvT": np.ascontiguousarray(Wv[fs, :].T).astype(BF),
            "woT": np.ascontiguousarray(Wo[:, fs].T).astype(BF),
            "cosT": cosT.astype(BF),
            "sinT": sinT.astype(BF),
            "mskT": msk.astype(BF),
        })
    return maps


def run(x, Wq, Wk, Wv, Wo, trace=False, **spmd_kwargs):
    """Run on 8 cores; returns (full_output, BassKernelResults)."""
    x = np.asarray(x, np.float32)
    Wq = np.asarray(Wq, np.float32)
    Wk = np.asarray(Wk, np.float32)
    Wv = np.asarray(Wv, np.float32)
    Wo = np.asarray(Wo, np.float32)
    nc = _get_nc()
    maps = _in_maps(x, Wq, Wk, Wv, Wo)
    res = bass_utils.run_bass_kernel_spmd(nc, maps, core_ids=list(range(8)),
                                          trace=trace, **spmd_kwargs)
    outs = [res.results[c]["out"].astype(np.float32) for c in range(8)]
    full = np.empty((2, S, D), np.float32)
    for b in range(2):
        # each core returns its o_proj partial TRANSPOSED ([D_out, S])
        acc = outs[4 * b] + outs[4 * b + 1] + outs[4 * b + 2] + outs[4 * b + 3]
        full[b] = acc.T
    return full, res


def kernel(x, Wq, Wk, Wv, Wo):
    full, _ = run(x, Wq, Wk, Wv, Wo)
    return full


# revision 32
# speedup vs baseline: 1.0338x; 1.0106x over previous
"""Multi-head attention (RoPE, causal, fp32) on 8 Trainium2 NeuronCores.

Problem: B=2, S=2048, D=2048, H=16 heads (hd=128).
Sharding: DP=2 (batch) x TP=4 (head groups of 4 heads). Core c handles
batch c//4, head group c%4. Each core computes q/k/v projections for its
512 features, RoPE, causal attention, and a partial o_proj against its
512 columns of Wo. The host sums the 4 partial o_proj outputs per batch.

Layout strategy (per core), all compute in bf16 with fp32 PSUM
accumulation (rel-err budget 2e-2; measured ~6.5e-3):
  - All inputs host-cast to bf16: halves HBM traffic and enables FWL
    fast weight loads. Per-chunk DMAs are spread across the Sync,
    Scalar and GpSimd engine queues — each dma_start lands on its own
    hardware ring, so many small transfers run in parallel and the
    ~0.6us/DMA issue rate never starves the PE.
  - Single fused pass over x: per 512-seq chunk, three j-loops (q all
    heads, k all heads, v) each accumulate a 4-bank PSUM tile; the
    "acc" tag rotates through 2 slots so evictions (ScalarE) always
    overlap the next loop's matmuls. x is read from HBM exactly once.
  - RoPE applied on the bf16 slabs after eviction (rowswap via
    SBUF->SBUF DMA on the GpSimd queue, sign baked into the host sin
    table), on VectorE.
  - Attention entirely in transposed space: scoresT[k, q] tiles,
    lhsT=kT slice, rhs=qT chunk, exp fused into the PSUM eviction on
    ScalarE. One global software pipeline across ALL (head, q-chunk)
    pairs — score j-pair steps run two steps ahead of the denominator/
    attn@V accumulation, with per-j-pair ex tiles so the pipeline never
    drains at pair boundaries.
  - Causal: only j <= q k-tiles computed; diagonal k-tiles compute only
    the alive (q >= k) column range (partial-N matmuls for score, den,
    attn@V and the exp eviction), one shared [128,128] triangular mask
    multiplied on the edge block.
  - Softmax denominator via all-ones [128,128] stationary matmul (k-sum
    pre-broadcast across partitions); 1/denom via one DVE
    reciprocal_approx_fast; normalization folded into the attn@V
    eviction.
  - Attention is q-chunk-major and the o_proj for each q-chunk runs as
    a burst right after its last head finishes. Burst accumulators
    borrow the 1-bank "pden" PSUM slots (denominators are consumed by
    then), leaving the "psc" slots free so the next q-block's score
    pipeline flows THROUGH the burst instead of restarting after it.
    Output stores start ~60us early; the kernel tail is only the last
    chunk's drain. Partials stored TRANSPOSED ([D_out,S]) in bf16; the
    host sums the 4 per-batch partials in fp32.

Known-dead ends (measured slower or incorrect on HW): col-tiled 32-wide
denominator strips (tile-mode-switch drains + nondeterministic results
next to fp32r matmuls), GpSimd partition_all_reduce denominators
(~3.7us per [128,512] — serializes each q-block), whole-tensor batched
DMAs (one ring per dma_start caps at ~35GB/s).
"""

import sys

for _p in ("/opt/trn_rl_repo",):
    if _p not in sys.path:
        sys.path.insert(0, _p)

import ml_dtypes
import numpy as np

import concourse.bass as bass
import concourse.mybir as mybir
import concourse.tile as tile
from concourse import bacc, bass_utils


# NOTE: the baseline's --enable-ldw-opt=true patch is incompatible with
# bf16 LDWEIGHTS (walrus rejects FWL loads under ldw-opt), so it is not
# used here. LDWEIGHTS issue is hidden under matmul streaming via the
# dual SBUF read ports, so the elision is not needed.

P = 128          # partitions / head dim
S = 2048         # sequence length
D = 2048         # model dim
F = 512          # features per core (4 heads)
H = 4            # heads per core
HD = 128         # head dim
NJ = D // P      # 16 contraction chunks of 128
NQ = S // 512    # 4 query chunks of 512
SCALE = 1.0 / float(np.sqrt(HD))

F32 = mybir.dt.float32
BF16 = mybir.dt.bfloat16
AFT = mybir.ActivationFunctionType


def _proj_phase(tc, xT, wqT, wkT, wvT, cosT, sinT, qT, kT, vN):
    """Fused q/k/v projections + RoPE: one pass over x.

    Per 512-seq chunk, three j-loops (q all heads, k all heads, v) each
    accumulate a 4-bank PSUM tile; the "acc" tag rotates through 2 slots
    (8 banks) so evictions always overlap the next loop's matmuls. At
    s==0 the input DMA is staggered so the q-loop only carries the
    tensors it needs (x chunk + Wq + half of Wk): the PE is compute-
    paced from the first matmul on.
    """
    nc = tc.nc
    # dram views with the j-chunk (128-row blocks) split out: one strided
    # DMA can then load a whole tensor (the Sync engine's ~0.6us per-DMA
    # issue rate is the s==0 bottleneck, not HBM bandwidth)
    xR = xT.rearrange("(j p) s -> p j s", p=P)
    wqR = wqT.rearrange("(j p) f -> p j f", p=P)
    wkR = wkT.rearrange("(j p) f -> p j f", p=P)
    wvR = wvT.rearrange("(j p) f -> p j f", p=P)
    with tc.tile_pool(name="cs", bufs=1) as cspool, \
         tc.tile_pool(name="w", bufs=1) as wpool, \
         tc.tile_pool(name="xs", bufs=2) as xspool, \
         tc.tile_pool(name="rope", bufs=4) as rpool, \
         tc.tile_pool(name="pp", bufs=1, space="PSUM") as pp:
        cos_sb = cspool.tile([P, S], BF16)
        sin_sb = cspool.tile([P, S], BF16)
        wq_sb = wpool.tile([P, NJ, F], BF16)
        wk_sb = wpool.tile([P, NJ, F], BF16)
        wv_sb = wpool.tile([P, NJ, F], BF16)

        xs_tiles = {}

        def load_xs(s):
            xs = xspool.tile([P, NJ, 512], BF16, name=f"xs{s}", tag="xs")
            xs_tiles[s] = xs
            return xs

        xs0 = load_xs(0)
        # per-j DMAs (each dma_start lands on its own hardware ring, so
        # many small transfers run in parallel) split across the three
        # DMA-capable engine queues to parallelize the ~0.6us-per-DMA
        # issue cost: Sync=x, Scalar=wq then wv, GpSimd=cos/sin then wk
        for j in range(NJ):
            xeng = nc.sync if j % 2 == 0 else nc.gpsimd
            xeng.dma_start(xs0[:, j, :], xR[:, j, 0:512])
            # j==0 wq rides the Sync queue: the Scalar queue's first DMA
            # sits behind the ~1.3us ACT_TABLE_LOAD preamble
            weng = nc.sync if j == 0 else nc.scalar
            weng.dma_start(wq_sb[:, j, :], wqR[:, j, :])
        nc.gpsimd.dma_start(cos_sb[:], cosT)
        nc.gpsimd.dma_start(sin_sb[:], sinT)
        for j in range(NJ):
            nc.gpsimd.dma_start(wk_sb[:, j, :], wkR[:, j, :])
            nc.scalar.dma_start(wv_sb[:, j, :], wvR[:, j, :])

        def rope(dst, sl, sh, h):
            # dst = dst*cos + rowswap(dst)*sin, in place on the slab (bf16)
            rt = rpool.tile([P, 512], BF16, name=f"rt{sh}_{h}", tag="rt")
            nc.gpsimd.dma_start(rt[0:64, :], dst[64:128, :])
            nc.gpsimd.dma_start(rt[64:128, :], dst[0:64, :])
            nc.vector.tensor_mul(rt[:], rt[:], sin_sb[:, sl])
            nc.vector.tensor_mul(dst, dst, cos_sb[:, sl])
            nc.vector.tensor_add(dst, dst, rt[:])

        for s in range(NQ):
            sl = slice(s * 512, (s + 1) * 512)
            xs = xs_tiles[s]
            # ---- q loop (all 4 heads) ----
            pq = pp.tile([P, H, 512], F32, name=f"pq{s}", tag="acc", bufs=2)
            for j in range(NJ):
                for h in range(H):
                    nc.tensor.matmul(pq[:, h, :],
                                     wq_sb[:, j, h * HD:(h + 1) * HD],
                                     xs[:, j, :],
                                     start=(j == 0), stop=(j == NJ - 1))

            nc.scalar.activation(qT[:, :, sl], pq[:], AFT.Copy)
            for h in range(H):
                rope(qT[:, h, sl], sl, s, f"q{h}")
            # ---- k loop (all 4 heads) ----
            pk = pp.tile([P, H, 512], F32, name=f"pk{s}", tag="acc", bufs=2)
            for j in range(NJ):
                for h in range(H):
                    nc.tensor.matmul(pk[:, h, :],
                                     wk_sb[:, j, h * HD:(h + 1) * HD],
                                     xs[:, j, :],
                                     start=(j == 0), stop=(j == NJ - 1))

            nc.scalar.activation(kT[:, :, sl], pk[:], AFT.Copy)
            for h in range(H):
                rope(kT[:, h, sl], sl, s, f"k{h}")
            # ---- v loop (x chunks become the stationaries) ----
            pv = pp.tile([P, H, 512], F32, name=f"pv{s}", tag="acc", bufs=2)
            for j in range(NJ):
                for st in range(4):
                    nc.tensor.matmul(pv[:, st, :],
                                     xs[:, j, st * P:(st + 1) * P],
                                     wv_sb[:, j, :],
                                     start=(j == 0), stop=(j == NJ - 1))
                if s < NQ - 1:
                    xsn = load_xs(s + 1) if j == 0 else xs_tiles[s + 1]
                    nc.sync.dma_start(xsn[:, j, :],
                                      xR[:, j, (s + 1) * 512:(s + 2) * 512])
            nc.scalar.activation(vN[:, 4 * s:4 * s + 4, :], pv[:], AFT.Copy)


def _attn_phase(tc, mskT, woT, qT, kT, vN, oT, wo_sb, out):
    """Causal attention, one global software pipeline over (h, q, j-pair)."""
    nc = tc.nc
    with tc.tile_pool(name="amsk", bufs=1, side="right") as mpool, \
         tc.tile_pool(name="exp", bufs=5, side="right") as epool, \
         tc.tile_pool(name="attsb", bufs=2, side="right") as apool, \
         tc.tile_pool(name="oev", bufs=3, side="right") as oevp, \
         tc.tile_pool(name="pa", bufs=1, space="PSUM") as pap:
        msk_sb = mpool.tile([P, P], BF16)
        nc.sync.dma_start(msk_sb[:], mskT)
        # all-ones [128,128] stationary: the denominator matmul then yields
        # the k-sum already broadcast across all 128 partitions of PSUM.
        ones_tmp = mpool.tile([P, P], F32)
        nc.vector.memset(ones_tmp[:], 1.0)
        ones_mat = mpool.tile([P, P], BF16)
        nc.vector.tensor_copy(ones_mat[:], ones_tmp[:])
        # prefetch Wo during attention compute (Scalar queue, per head)
        woR = woT.rearrange("(h p) d -> p h d", p=P)
        for h in range(H):
            nc.scalar.dma_start(wo_sb[:, h, :], woR[:, h, :])

        # ---- flattened list of score/acc steps, q-major so each
        # q-chunk's o_proj burst can run as soon as all heads finish ----
        steps = []
        for q in range(NQ):
            for h in range(H):
                jmax = 4 * (q + 1)
                for jj in range(0, jmax, 2):
                    steps.append((h, q, jj, jmax))
        state = {}  # (h,q) -> dict with psum tiles + ex tiles per step

        def score_step(i):
            h, q, jj, jmax = steps[i]
            qh = qT[:, h, :]
            kh = kT[:, h, :]
            ex = epool.tile([P, 2, 512], BF16, name=f"ex{h}_{q}_{jj}", tag="ex")
            psc = pap.tile([P, 2, 512], F32, name=f"psc{h}{q}{jj}",
                           tag="psc", bufs=2)
            offs = []
            for t in range(2):
                j = jj + t
                dd = j - 4 * q
                off = dd * P if dd > 0 else 0
                offs.append(off)
                nc.tensor.matmul(psc[:, t, off:512],
                                 kh[:, j * P:(j + 1) * P],
                                 qh[:, q * 512 + off:(q + 1) * 512],
                                 start=True, stop=True)
            if offs[0] == offs[1]:
                # same width: one fused exp eviction for the pair
                nc.scalar.activation(ex[:, 0:2, offs[0]:512],
                                     psc[:, 0:2, offs[0]:512],
                                     AFT.Exp, scale=SCALE)
            else:
                for t in range(2):
                    nc.scalar.activation(ex[:, t, offs[t]:512],
                                         psc[:, t, offs[t]:512],
                                         AFT.Exp, scale=SCALE)
            # triangular mask on the diagonal 128-block
            for t in range(2):
                j = jj + t
                dd = j - 4 * q
                if dd >= 0:
                    nc.vector.tensor_mul(
                        ex[:, t, dd * P:(dd + 1) * P],
                        ex[:, t, dd * P:(dd + 1) * P], msk_sb[:])
            # den-group pre-sums issued at score time so the grouped
            # denominator matmul (two steps later) never waits on VectorE
            if q > 0 and jj < jmax - 4:
                if jj % 4 == 0:
                    gacc = epool.tile([P, 512], BF16, name=f"ga{h}{q}{jj}",
                                      tag="gacc", bufs=2)
                    state[(h, q, 'g', jj // 4)] = gacc
                    nc.vector.tensor_add(gacc[:], ex[:, 0, :], ex[:, 1, :])
                else:
                    gacc = state[(h, q, 'g', jj // 4)]
                    nc.vector.tensor_add(gacc[:], gacc[:], ex[:, 0, :])
                    nc.vector.tensor_add(gacc[:], gacc[:], ex[:, 1, :])
            state[(h, q, jj)] = ex

        def acc_step(i):
            h, q, jj, jmax = steps[i]
            ex = state.pop((h, q, jj))
            key = (h, q)
            if jj == 0:
                pden = pap.tile([P, 512], F32, name=f"pden{h}{q}",
                                tag="pden", bufs=2)
                pov = pap.tile([P, 512], F32, name=f"pov{h}{q}",
                               tag="pov", bufs=2)
                state[key] = (pden, pov)
            pden, pov = state[key]
            for t in range(2):
                j = jj + t
                dd = j - 4 * q
                off = dd * P if dd > 0 else 0
                nc.tensor.matmul(pov[:, off:512],
                                 vN[:, j, h * HD:(h + 1) * HD],
                                 ex[:, t, off:512],
                                 start=(j == 0), stop=(j == jmax - 1))
            # denominator: the ones matmul is linear in ex and shares its
            # stationary, so 4 full-width ex chunks were pre-summed on
            # VectorE at score time and feed ONE matmul here. Diagonal
            # (partial-N) chunks and q==0 go straight to per-chunk
            # partial matmuls.
            if q == 0 or jj >= jmax - 4:
                for t in range(2):
                    j = jj + t
                    dd = j - 4 * q
                    off = dd * P if dd > 0 else 0
                    nc.tensor.matmul(pden[:, off:512], ones_mat[:],
                                     ex[:, t, off:512],
                                     start=(j == 0), stop=(j == jmax - 1))
            elif jj % 4 == 2:
                gacc = state.pop((h, q, 'g', jj // 4))
                nc.tensor.matmul(pden[:], ones_mat[:], gacc[:],
                                 start=(jj == 2), stop=False)
            if jj == jmax - 2:
                del state[key]
                rbc = apool.tile([P, 512], F32, name=f"rbc{h}{q}", tag="rbc")
                nc.vector.reciprocal_approx_fast(rbc[:], pden[:])
                nc.vector.tensor_mul(
                    oT[:, h, q * 512:(q + 1) * 512],
                    pov[:], rbc[:])

        def oproj_burst(qc):
            # o_proj for this q-chunk: weight-stationary over 16 dt blocks,
            # PSUM shared with the score tiles ("psc" tag), evictions on
            # VectorE (ScalarE is saturated by exp), bf16 stores
            qsl = slice(qc * 512, (qc + 1) * 512)
            # po tiles borrow the "pden" slots (denominators for this
            # block are already consumed), NOT the "psc" slots: the next
            # q-block's score pipeline keeps flowing through the burst
            # instead of restarting after it.
            for dt in range(D // P):
                po = pap.tile([P, 512], F32, name=f"po{qc}_{dt}",
                              tag="pden", bufs=2)
                for h in range(H):
                    nc.tensor.matmul(po[:],
                                     wo_sb[:, h, dt * P:(dt + 1) * P],
                                     oT[:, h, qsl],
                                     start=(h == 0), stop=(h == H - 1))
                ot = oevp.tile([P, 512], BF16, name=f"ot{qc}_{dt}",
                               tag="ot")
                if qc == NQ - 1 and dt % 2 == 1:
                    # last q-chunk: exp is finished, ScalarE is free to
                    # share eviction duty and shorten the tail
                    nc.scalar.activation(ot[:], po[:], AFT.Copy)
                else:
                    nc.vector.tensor_copy(ot[:], po[:])
                # stores alternate between the Sync and Scalar DMA queues
                eng = nc.sync if dt % 2 == 0 else nc.scalar
                eng.dma_start(out[dt * P:(dt + 1) * P, qsl], ot[:])

        def acc_and_maybe_burst(i):
            acc_step(i)
            h, q, jj, jmax = steps[i]
            if h == H - 1 and jj == jmax - 2:
                oproj_burst(q)

        # software pipeline: scores two steps ahead of accumulation
        n = len(steps)
        for i in range(n):
            score_step(i)
            if i >= 2:
                acc_and_maybe_burst(i - 2)
        acc_and_maybe_burst(n - 2)
        acc_and_maybe_burst(n - 1)


def _body(tc, xT, wqT, wkT, wvT, woT, cosT, sinT, mskT, out):
    nc = tc.nc
    # long-lived slabs; left stack for qkv, right for attention-era tensors
    p_qk = tc.alloc_tile_pool(name="p_qk", bufs=1, side="left")
    qT = p_qk.tile([P, H, S], BF16)   # [hd, head, seq]
    kT = p_qk.tile([P, H, S], BF16)
    p_v = tc.alloc_tile_pool(name="p_v", bufs=1, side="left")
    vN = p_v.tile([P, NJ, F], BF16)   # [:, j, :] = v[j*128:(j+1)*128, :]

    _proj_phase(tc, xT, wqT, wkT, wvT, cosT, sinT, qT, kT, vN)

    p_oT = tc.alloc_tile_pool(name="p_oT", bufs=1, side="right")
    oT = p_oT.tile([P, H, S], BF16)   # attention output, transposed
    p_wo = tc.alloc_tile_pool(name="p_wo", bufs=1, side="right")
    wo_sb = p_wo.tile([P, H, D], BF16)

    _attn_phase(tc, mskT, woT, qT, kT, vN, oT, wo_sb, out)

    p_v.release()
    p_qk.release()
    p_wo.release()
    p_oT.release()


def build_nc():
    nc = bacc.Bacc("TRN2", target_bir_lowering=False, debug=False,
                   enable_asserts=True, num_devices=8)
    xT = nc.dram_tensor("xT", [D, S], BF16, kind="ExternalInput").ap()
    wqT = nc.dram_tensor("wqT", [D, F], BF16, kind="ExternalInput").ap()
    wkT = nc.dram_tensor("wkT", [D, F], BF16, kind="ExternalInput").ap()
    wvT = nc.dram_tensor("wvT", [D, F], BF16, kind="ExternalInput").ap()
    woT = nc.dram_tensor("woT", [F, D], BF16, kind="ExternalInput").ap()
    cosT = nc.dram_tensor("cosT", [P, S], BF16, kind="ExternalInput").ap()
    sinT = nc.dram_tensor("sinT", [P, S], BF16, kind="ExternalInput").ap()
    mskT = nc.dram_tensor("mskT", [P, P], BF16, kind="ExternalInput").ap()
    out = nc.dram_tensor("out", [S, D], BF16, kind="ExternalOutput").ap()

    with tile.TileContext(nc) as tc:
        _body(tc, xT, wqT, wkT, wvT, woT, cosT, sinT, mskT, out)
    nc.compile()
    return nc


_CACHE = {}


def _get_nc():
    if "nc" not in _CACHE:
        _CACHE["nc"] = build_nc()
    return _CACHE["nc"]


def _rope_tables():
    hd = HD
    inv = 1.0 / (10000.0 ** (np.arange(0, hd, 2, dtype=np.float32) / np.float32(hd)))
    t = np.arange(S, dtype=np.float32)
    freqs = np.outer(t, inv)                      # [S, 64]
    emb = np.concatenate([freqs, freqs], axis=-1)  # [S, 128]
    cosT = np.cos(emb).T.astype(np.float32).copy()
    sinT = np.sin(emb).T.astype(np.float32).copy()
    sinT[0:64, :] *= -1.0  # sign of rotate_half baked into the table
    return np.ascontiguousarray(cosT), np.ascontiguousarray(sinT)


def _diag_masks():
    kp = np.arange(P)[:, None]
    qf = np.arange(P)[None, :]
    return np.ascontiguousarray((kp <= qf).astype(np.float32))


def _in_maps(x, Wq, Wk, Wv, Wo):
    cosT, sinT = _rope_tables()
    msk = _diag_masks()
    BF = ml_dtypes.bfloat16
    maps = []
    for c in range(8):
        b, g = c // 4, c % 4
        fs = slice(g * F, (g + 1) * F)
        maps.append({
            "xT": np.ascontiguousarray(x[b].T).astype(BF),
            "wqT": np.ascontiguousarray(Wq[fs, :].T).astype(BF),
            "wkT": np.ascontiguousarray(Wk[fs, :].T).astype(BF),
            "wvT": np.ascontiguousarray(Wv[fs, :].T).astype(BF),
            "woT": np.ascontiguousarray(Wo[:, fs].T).astype(BF),
            "cosT": cosT.astype(BF),
            "sinT": sinT.astype(BF),
            "mskT": msk.astype(BF),
        })
    return maps


def run(x, Wq, Wk, Wv, Wo, trace=False, **spmd_kwargs):
    """Run on 8 cores; returns (full_output, BassKernelResults)."""
    x = np.asarray(x, np.float32)
    Wq = np.asarray(Wq, np.float32)
    Wk = np.asarray(Wk, np.float32)
    Wv = np.asarray(Wv, np.float32)
    Wo = np.asarray(Wo, np.float32)
    nc = _get_nc()
    maps = _in_maps(x, Wq, Wk, Wv, Wo)
    res = bass_utils.run_bass_kernel_spmd(nc, maps, core_ids=list(range(8)),
                                          trace=trace, **spmd_kwargs)
    outs = [res.results[c]["out"].astype(np.float32) for c in range(8)]
    full = np.empty((2, S, D), np.float32)
    for b in range(2):
        # each core returns its o_proj partial TRANSPOSED ([D_out, S])
        acc = outs[4 * b] + outs[4 * b + 1] + outs[4 * b + 2] + outs[4 * b + 3]
        full[b] = acc.T
    return full, res


def kernel(x, Wq, Wk, Wv, Wo):
    full, _ = run(x, Wq, Wk, Wv, Wo)
    return full
